# revision 17
# baseline (speedup 1.0000x reference)
"""Trainium2 Bass kernel for nn_MetaRL_LightGAT_BiACT (GAT + LayerNorm + MLP).

Strategy (8 NeuronCores, row-sharded, indicator-split formulation):

  exp(leaky_relu(s_i + s_j)) is exactly u_i*v_j when s_i+s_j > 0 and
  w_i*z_j otherwise, where u=exp(s), w=exp(0.2 s) (v=u, z=w over j).
  With c_ij = 1[s_i+s_j > 0] and A1 = adj*c, the GAT aggregation
  numerator (and denominator, via a ones column) becomes

     num_i = u_i * (A1 @ vWh)_i + w_i * ((adj @ zWh)_i - (A1 @ zWh)_i)

  i.e. two plain matmuls per j-chunk with 0/1 rhs masks -- no exp or
  leaky-relu over the N^2 data at all.

  Host precomputes a single pre-transposed fp16 slab
     slabG[j, i] = s_i + 4*(adj[i,j] - 1)
  from which BOTH masks fall out as one tensor_scalar each (4x DVE mode):
     A1 = (slabG + s_j) > 0        (adj=0 entries are < -2, never pass)
     A0 = slabG > -2               (recovers adj)

  Per j-chunk (128 j's x 1024 i's) on each core:
    DMA:  slabG chunk [128, 1024] fp16 (plain contiguous load)
    DVE:  A1 = ts(slabG add s_j, is_gt 0) -> bf16   (4x mode)
          A0 = ts(slabG is_gt -2)         -> bf16   (4x mode)
    PE:   accCat[0:128]  += [vWh | -zWh]_chunk^T @ A1   (bf16, 1 cyc/row)
          accCat[64:128] += zWh_chunk^T @ A0            (same PSUM bank;
                            accumulates zWh@(A0-A1) in rows 64..112)
  Epilogue: shift accCat[64:113] down via tiny DMA, combine with u/w,
  divide by denominator row, LayerNorm (f32), MLP 48->256->128->32 in
  bf16 on PE, transpose out.
"""

import sys

if "/opt/trn_rl_repo" not in sys.path:
    sys.path.insert(0, "/opt/trn_rl_repo")

import numpy as np

N = 8192
D_IN = 128
D_H = 48
D_OUT = 32
N_CORES = 8
ROWS = N // N_CORES          # 1024 rows (i) per core
P = 128                      # partitions
NEG_SLOPE = 0.2
EPS = 1e-5
MOFF = 60.0                  # mask offset folded into slabG


def build_nc(num_cores=N_CORES, rows=ROWS, n=N, slab_bufs=3, mask_bufs=2,
             reps=1, exp_chunks=32, lrelu_mode="act",
             stages="dma,cmp,mm,epi"):
    import concourse.bass as bass
    import concourse.mybir as mybir
    import concourse.tile as tile
    from concourse import bacc
    from concourse.masks import make_identity
    from contextlib import ExitStack

    f32 = mybir.dt.float32
    f16 = mybir.dt.float16
    bf16 = mybir.dt.bfloat16
    AF = mybir.ActivationFunctionType
    OP = mybir.AluOpType

    n_chunk = n // P             # j-chunks
    n_half = rows // 512         # 512-wide i halves

    st = {}
    for tok in stages.split(","):
        name, _, mult = tok.partition(":")
        st[name] = int(mult) if mult else 1
    nc = bacc.Bacc("TRN2", target_bir_lowering=False, debug=False,
                   num_devices=num_cores)

    slab_d = nc.dram_tensor("slabg", [n, rows], bf16, kind="ExternalInput").ap()
    catwh_d = nc.dram_tensor("catwh", [n, P], bf16, kind="ExternalInput").ap()
    zwh_d = nc.dram_tensor("zwh", [n, 64], bf16, kind="ExternalInput").ap()
    whaug_d = nc.dram_tensor("whaug", [n, 64], bf16, kind="ExternalInput").ap()
    sP_d = nc.dram_tensor("sP", [P, n_chunk], f32, kind="ExternalInput").ap()
    sPn_d = nc.dram_tensor("sPn", [P, n_chunk], f32, kind="ExternalInput").ap()
    uw_d = nc.dram_tensor("uw", [2, rows], f32, kind="ExternalInput").ap()
    gamma_d = nc.dram_tensor("gamma", [1, D_H], f32, kind="ExternalInput").ap()
    beta_d = nc.dram_tensor("beta", [1, D_H], f32, kind="ExternalInput").ap()
    w1t_d = nc.dram_tensor("w1t", [D_H, 256], bf16, kind="ExternalInput").ap()
    b1_d = nc.dram_tensor("b1", [256, 1], f32, kind="ExternalInput").ap()
    w2t_d = nc.dram_tensor("w2t", [256, 128], bf16, kind="ExternalInput").ap()
    b2_d = nc.dram_tensor("b2", [128, 1], f32, kind="ExternalInput").ap()
    w3t_d = nc.dram_tensor("w3t", [128, D_OUT], bf16, kind="ExternalInput").ap()
    b3_d = nc.dram_tensor("b3", [D_OUT, 1], f32, kind="ExternalInput").ap()
    out_d = nc.dram_tensor("out", [rows, D_OUT], f32, kind="ExternalOutput").ap()

    with ExitStack() as ctx:
        tc = ctx.enter_context(tile.TileContext(nc))
        singles = ctx.enter_context(tc.tile_pool(name="singles", bufs=1))
        slabp = ctx.enter_context(tc.tile_pool(name="slabp", bufs=slab_bufs))
        maskp = ctx.enter_context(tc.tile_pool(name="maskp", bufs=mask_bufs))
        hp = ctx.enter_context(tc.tile_pool(name="hp", bufs=2))

        # ---- resident small tensors ----
        catwh_sb = singles.tile([P, n_chunk, P], bf16)
        nc.sync.dma_start(catwh_sb, catwh_d.rearrange("(c p) m -> p c m", p=P))
        zwh_sb = singles.tile([P, n_chunk, 64], bf16)
        nc.sync.dma_start(zwh_sb, zwh_d.rearrange("(c p) m -> p c m", p=P))
        sP_sb = singles.tile([P, n_chunk], f32)
        nc.sync.dma_start(sP_sb, sP_d)
        sPn_sb = singles.tile([P, n_chunk], f32)
        nc.sync.dma_start(sPn_sb, sPn_d)
        sP2_sb = singles.tile([P, n_chunk], f32)
        nc.scalar.activation(sP2_sb, sP_sb, AF.Copy, scale=NEG_SLOPE)
        whaug_sb = singles.tile([P, n_chunk, 64], bf16)
        nc.sync.dma_start(whaug_sb, whaug_d.rearrange("(c p) m -> p c m", p=P))
        # u replicated over partitions 0..63, w over all 128 (used at 64:113)
        u_rep = singles.tile([64, rows], f32)
        nc.sync.dma_start(u_rep, uw_d[0:1, :].partition_broadcast(64)
                          .rearrange("p one r -> p (one r)"))
        w_rep = singles.tile([P, rows], f32)
        nc.sync.dma_start(w_rep, uw_d[1:2, :].partition_broadcast(P)
                          .rearrange("p one r -> p (one r)"))
        gammaC = singles.tile([D_H, 1], f32)
        nc.sync.dma_start(gammaC, gamma_d.rearrange("one d -> d one"))
        betaC = singles.tile([D_H, 1], f32)
        nc.sync.dma_start(betaC, beta_d.rearrange("one d -> d one"))
        w1t_sb = singles.tile([D_H, 256], bf16)
        nc.sync.dma_start(w1t_sb, w1t_d)
        w2t_sb = singles.tile([P, 2, 128], bf16)
        nc.sync.dma_start(w2t_sb, w2t_d.rearrange("(m p) k -> p m k", p=P))
        w3t_sb = singles.tile([P, D_OUT], bf16)
        nc.sync.dma_start(w3t_sb, w3t_d)
        b1_sb = singles.tile([P, 2], f32)
        nc.sync.dma_start(b1_sb, b1_d.rearrange("(m p) one -> p (m one)", p=P))
        b2_sb = singles.tile([P, 1], f32)
        nc.sync.dma_start(b2_sb, b2_d)
        b3_sb = singles.tile([D_OUT, 1], f32)
        nc.sync.dma_start(b3_sb, b3_d)
        eps_sb = singles.tile([P, 1], f32)
        nc.vector.memset(eps_sb, EPS)
        ones48 = singles.tile([D_H, 1], bf16)
        nc.vector.memset(ones48, 1.0)
        ident = singles.tile([P, P], f32)
        make_identity(nc, ident)

        def bcast_sb(dst, src_row, parts, eng=None):
            src = bass.AP(tensor=src_row.tensor, offset=src_row.offset,
                          ap=[src_row.ap[0], [0, parts], src_row.ap[1]])
            dst3 = bass.AP(tensor=dst.tensor, offset=dst.offset,
                           ap=[dst.ap[0], [1, 1], dst.ap[1]])
            (eng or nc.sync).dma_start(dst3, src)

        slab_r = slab_d.rearrange("(c p) i -> p c i", p=P)

        # chunk classes: E (exp path on ACT, 1 matmul arm) / D (indicator
        # path on DVE, 2 matmul arms). Interleave E chunks for overlap.
        n_e = min(exp_chunks * n_chunk // 64, n_chunk)
        is_e = [False] * n_chunk
        cnt = 0
        for cc in range(n_chunk):
            if cnt < n_e and cc % 2 == 1:
                is_e[cc] = True
                cnt += 1
        for cc in range(n_chunk):
            if cnt < n_e and not is_e[cc]:
                is_e[cc] = True
                cnt += 1
        e_idx = [cc for cc in range(n_chunk) if is_e[cc]]
        d_idx = [cc for cc in range(n_chunk) if not is_e[cc]]
        for rep in range(reps):
          with tc.tile_pool(name=f"accp{rep}", bufs=2, space="PSUM") as accp:
            acc = [accp.tile([P, 512], f32, tag="acc", name=f"acc{h}")
                   for h in range(n_half)]
            accE = [accp.tile([64, 512], f32, tag="accE", name=f"accE{h}")
                    for h in range(n_half)] if e_idx else []
            for cc in range(n_chunk):
                slab = slabp.tile([P, rows], bf16, tag="slab")
                for _m in range(st.get("dma", 0)):
                    nc.sync.dma_start(slab, slab_r[:, cc, :])
                n_mm = st.get("mm", 0)
                if is_e[cc]:
                    te = maskp.tile([P, rows], bf16, tag="a1")
                    pe_ = maskp.tile([P, rows], bf16, tag="a0")
                    for _m in range(st.get("cmp", 0)):
                        if lrelu_mode == "act":
                            nc.scalar.activation(te, slab, AF.Prelu,
                                                 bias=sP_sb[:, cc:cc + 1],
                                                 alpha=NEG_SLOPE)
                            nc.scalar.activation(pe_, te, AF.Exp)
                        else:  # exp(leaky(x)) == max(exp(x), exp(0.2 x))
                            nc.scalar.activation(te, slab, AF.Exp,
                                                 scale=NEG_SLOPE,
                                                 bias=sP2_sb[:, cc:cc + 1])
                            nc.scalar.activation(pe_, slab, AF.Exp,
                                                 bias=sP_sb[:, cc:cc + 1])
                            nc.vector.tensor_tensor(pe_, pe_, te, OP.max)
                    for _m in range(n_mm):
                        for h in range(n_half):
                            sl = slice(h * 512, (h + 1) * 512)
                            nc.tensor.matmul(
                                accE[h][:, :],
                                lhsT=whaug_sb[:, cc, :],
                                rhs=pe_[:, sl],
                                start=(cc == e_idx[0] and _m == 0),
                                stop=(cc == e_idx[-1] and _m == n_mm - 1),
                                skip_group_check=True)
                    continue
                a1 = maskp.tile([P, rows], bf16, tag="a1")
                a0 = maskp.tile([P, rows], bf16, tag="a0")
                for _m in range(st.get("cmp", 0)):
                    nc.vector.tensor_scalar(a1, slab, sPn_sb[:, cc:cc + 1],
                                            None, OP.is_gt)
                    nc.vector.tensor_scalar(a0, slab, -(MOFF / 2), None,
                                            OP.is_gt)
                for _m in range(n_mm):
                    for h in range(n_half):
                        sl = slice(h * 512, (h + 1) * 512)
                        nc.tensor.matmul(
                            acc[h][:, :],
                            lhsT=catwh_sb[:, cc, :],
                            rhs=a1[:, sl],
                            start=(cc == d_idx[0] and _m == 0), stop=False,
                            skip_group_check=True)
                    for h in range(n_half):
                        sl = slice(h * 512, (h + 1) * 512)
                        nc.tensor.matmul(
                            acc[h][64:128, :],
                            lhsT=zwh_sb[:, cc, :],
                            rhs=a0[:, sl],
                            start=False,
                            stop=(cc == d_idx[-1] and _m == n_mm - 1),
                            skip_group_check=True)

            # ---- epilogue phase 1: combine, divide, LayerNorm ----
            hs = []
            do_epi = st.get("epi", 0) > 0 and st.get("mm", 0) > 0
            for h in range(n_half if do_epi else 0):
                sl = slice(h * 512, (h + 1) * 512)
                # combine: acc rows 0:49 hold vWh@A1 (u side), rows 64:113
                # hold zWh@(A0-A1) (w side). Weight each in place in PSUM,
                # bounce to SBUF, shift the w side down 64 partitions via
                # DMA, add, then divide by the denominator row.
                numT = hp.tile([49, 512], f32, tag="numT")
                if d_idx:
                    nc.vector.tensor_tensor(acc[h][64:113, :],
                                            acc[h][64:113, :],
                                            w_rep[64:113, sl], OP.mult)
                    nc.vector.tensor_tensor(acc[h][0:49, :], acc[h][0:49, :],
                                            u_rep[0:49, sl], OP.mult)
                    comb = hp.tile([P, 512], f32, tag="comb")
                    nc.scalar.activation(comb, acc[h][:, :], AF.Copy)
                    nb = hp.tile([49, 512], f32, tag="nb")
                    nc.scalar.dma_start(nb, comb[64:113, :])
                    nc.vector.tensor_tensor(numT, comb[0:49, :], nb, OP.add)
                    if e_idx:
                        nc.vector.tensor_tensor(numT, numT, accE[h][0:49, :],
                                                OP.add)
                else:
                    nc.vector.tensor_copy(numT, accE[h][0:49, :])
                den0 = hp.tile([1, 512], f32, tag="den0")
                nc.scalar.dma_start(den0, numT[48:49, :])
                rec = hp.tile([1, 512], f32, tag="rec")
                nc.vector.reciprocal_approx_fast(rec, den0)
                rbc = hp.tile([D_H, 512], f32, tag="rbc")
                bcast_sb(rbc, rec[0:1, :], D_H, eng=nc.scalar)
                hT = hp.tile([D_H, 512], f32, tag="hT", bufs=n_half)
                nc.vector.tensor_tensor(hT, numT[0:D_H, :], rbc, OP.mult)
                hT16 = hp.tile([D_H, 512], bf16, tag="hT16")
                nc.vector.tensor_copy(hT16, hT)
                sq = hp.tile([D_H, 512], bf16, tag="sq")
                nc.scalar.activation(sq, hT16, AF.Square)
                ssum = accp.tile([1, 512], f32, tag="ssum", name="ssum")
                nc.tensor.matmul(ssum, lhsT=ones48, rhs=hT16,
                                 start=True, stop=True)
                ssq = accp.tile([1, 512], f32, tag="ssq", name="ssq")
                nc.tensor.matmul(ssq, lhsT=ones48, rhs=sq,
                                 start=True, stop=True)
                mean = hp.tile([1, 512], f32, tag="mean")
                nc.scalar.activation(mean, ssum, AF.Copy, scale=1.0 / D_H)
                var = hp.tile([1, 512], f32, tag="var")
                nc.scalar.activation(var, ssq, AF.Copy, scale=1.0 / D_H,
                                     bias=EPS)
                msq = hp.tile([1, 512], f32, tag="msq")
                nc.vector.tensor_tensor(msq, mean, mean, OP.mult)
                nc.vector.tensor_tensor(var, var, msq, OP.subtract)
                lnv = hp.tile([1, 512], f32, tag="lnv")
                nc.scalar.activation(lnv, var, AF.Ln)
                rstd = hp.tile([1, 512], f32, tag="rstd")
                nc.scalar.activation(rstd, lnv, AF.Exp, scale=-0.5)
                mbc = hp.tile([D_H, 512], f32, tag="mbc")
                bcast_sb(mbc, mean[0:1, :], D_H, eng=nc.scalar)
                sbc = hp.tile([D_H, 512], f32, tag="sbc")
                bcast_sb(sbc, rstd[0:1, :], D_H, eng=nc.scalar)
                nc.vector.tensor_tensor(hT, hT, mbc, OP.subtract)
                nc.vector.tensor_tensor(hT, hT, sbc, OP.mult)
                hTb = hp.tile([D_H, 512], bf16, tag="hTb", bufs=n_half)
                nc.vector.tensor_scalar(hTb, hT, gammaC, betaC,
                                        OP.mult, OP.add)
                hs.append(hTb)

          # ---- epilogue phase 2: MLP head in transposed layout (bf16) ----
          with tc.tile_pool(name=f"mlpp{rep}", bufs=1, space="PSUM") as mlpp:
            for h in range(n_half if do_epi else 0):
                h1 = hp.tile([P, 2, 512], bf16, tag="h1")
                for m in range(2):
                    m1 = mlpp.tile([P, 512], f32, tag="m1")
                    nc.tensor.matmul(m1, lhsT=w1t_sb[:, m * P:(m + 1) * P],
                                     rhs=hs[h], start=True, stop=True)
                    nc.scalar.activation(h1[:, m, :], m1, AF.Relu,
                                         bias=b1_sb[:, m:m + 1])
                m2 = mlpp.tile([P, 512], f32, tag="m2")
                for m in range(2):
                    nc.tensor.matmul(m2, lhsT=w2t_sb[:, m, :],
                                     rhs=h1[:, m, :],
                                     start=(m == 0), stop=(m == 1))
                h2 = hp.tile([P, 512], bf16, tag="h2")
                nc.scalar.activation(h2, m2, AF.Relu, bias=b2_sb)
                m3 = mlpp.tile([D_OUT, 512], f32, tag="m3")
                nc.tensor.matmul(m3, lhsT=w3t_sb, rhs=h2,
                                 start=True, stop=True)
                h3 = hp.tile([D_OUT, 512], f32, tag="h3")
                nc.scalar.activation(h3, m3, AF.Identity, bias=b3_sb)
                for k in range(4):
                    ko = h * 4 + k
                    m4 = mlpp.tile([P, D_OUT], f32, tag="m4")
                    nc.tensor.transpose(m4, h3[:, k * P:(k + 1) * P],
                                        ident[0:D_OUT, 0:D_OUT])
                    ob = hp.tile([P, D_OUT], f32, tag="ob")
                    nc.vector.tensor_copy(ob, m4)
                    nc.scalar.dma_start(out_d[ko * P:(ko + 1) * P, :], ob)

    nc.compile()
    return nc


def host_prep(x, adj, W_gat, a, gamma, beta, W1, b1, W2, b2, W3, b3,
              num_cores=N_CORES):
    import ml_dtypes

    bf16 = ml_dtypes.bfloat16
    n = x.shape[0]
    rows = n // num_cores
    n_chunk = n // P
    Wh = (x.astype(np.float32) @ W_gat.T.astype(np.float32))
    s = (Wh @ a.T.astype(np.float32)).ravel().astype(np.float32)
    assert np.abs(s).max() < MOFF / 2 - 0.1, "s out of slab-offset range"
    u = np.exp(s).astype(np.float32)          # exp(s)
    w = np.exp(NEG_SLOPE * s).astype(np.float32)
    # catwh: [vWh(48) v 0*15 | -zWh(48) -z 0*15]
    catwh = np.zeros((n, P), np.float32)
    catwh[:, 0:D_H] = u[:, None] * Wh
    catwh[:, D_H] = u
    catwh[:, 64:64 + D_H] = -(w[:, None] * Wh)
    catwh[:, 64 + D_H] = -w
    zwh = np.zeros((n, 64), np.float32)
    zwh[:, 0:D_H] = w[:, None] * Wh
    zwh[:, D_H] = w
    whaug = np.zeros((n, 64), np.float32)
    whaug[:, 0:D_H] = Wh
    whaug[:, D_H] = 1.0
    sP = np.ascontiguousarray(s.reshape(n_chunk, P).T)
    in_maps = []
    for c in range(num_cores):
        r = slice(c * rows, (c + 1) * rows)
        slabg = (s[r][None, :] +
                 MOFF * (adj[r].T.astype(np.float32) - 1.0)
                 ).astype(bf16)
        in_maps.append({
            "slabg": np.ascontiguousarray(slabg),
            "catwh": catwh.astype(bf16),
            "zwh": zwh.astype(bf16),
            "whaug": whaug.astype(bf16),
            "sP": sP,
            "sPn": np.ascontiguousarray(-sP),
            "uw": np.ascontiguousarray(np.stack([u[r], w[r]])),
            "gamma": np.ascontiguousarray(gamma[None, :]).astype(np.float32),
            "beta": np.ascontiguousarray(beta[None, :]).astype(np.float32),
            "w1t": np.ascontiguousarray(W1.T).astype(bf16),
            "b1": np.ascontiguousarray(b1[:, None]).astype(np.float32),
            "w2t": np.ascontiguousarray(W2.T).astype(bf16),
            "b2": np.ascontiguousarray(b2[:, None]).astype(np.float32),
            "w3t": np.ascontiguousarray(W3.T).astype(bf16),
            "b3": np.ascontiguousarray(b3[:, None]).astype(np.float32),
        })
    return in_maps


_NC_CACHE = {}


def kernel(x, adj, W_gat, a, gamma, beta, W1, b1, W2, b2, W3, b3,
           trace=False):
    from concourse.bass_utils import run_bass_kernel_spmd

    args = [np.asarray(t) for t in
            (x, adj, W_gat, a, gamma, beta, W1, b1, W2, b2, W3, b3)]
    in_maps = host_prep(*args)
    if "nc" not in _NC_CACHE:
        _NC_CACHE["nc"] = build_nc()
    nc = _NC_CACHE["nc"]
    res = run_bass_kernel_spmd(nc, in_maps, list(range(N_CORES)), trace=trace)
    out = np.concatenate([r["out"] for r in res.results], axis=0)
    if trace:
        kernel.last_results = res
    return out.astype(np.float32)


# revision 18
# speedup vs baseline: 1.0375x; 1.0375x over previous
"""Trainium2 Bass kernel for nn_MetaRL_LightGAT_BiACT (GAT + LayerNorm + MLP).

Strategy (8 NeuronCores, row-sharded, indicator-split formulation):

  exp(leaky_relu(s_i + s_j)) is exactly u_i*v_j when s_i+s_j > 0 and
  w_i*z_j otherwise, where u=exp(s), w=exp(0.2 s) (v=u, z=w over j).
  With c_ij = 1[s_i+s_j > 0] and A1 = adj*c, the GAT aggregation
  numerator (and denominator, via a ones column) becomes

     num_i = u_i * (A1 @ vWh)_i + w_i * ((adj @ zWh)_i - (A1 @ zWh)_i)

  i.e. two plain matmuls per j-chunk with 0/1 rhs masks -- no exp or
  leaky-relu over the N^2 data at all.

  Host precomputes a single pre-transposed fp16 slab
     slabG[j, i] = s_i + 4*(adj[i,j] - 1)
  from which BOTH masks fall out as one tensor_scalar each (4x DVE mode):
     A1 = (slabG + s_j) > 0        (adj=0 entries are < -2, never pass)
     A0 = slabG > -2               (recovers adj)

  Per j-chunk (128 j's x 1024 i's) on each core:
    DMA:  slabG chunk [128, 1024] fp16 (plain contiguous load)
    DVE:  A1 = ts(slabG add s_j, is_gt 0) -> bf16   (4x mode)
          A0 = ts(slabG is_gt -2)         -> bf16   (4x mode)
    PE:   accCat[0:128]  += [vWh | -zWh]_chunk^T @ A1   (bf16, 1 cyc/row)
          accCat[64:128] += zWh_chunk^T @ A0            (same PSUM bank;
                            accumulates zWh@(A0-A1) in rows 64..112)
  Epilogue: shift accCat[64:113] down via tiny DMA, combine with u/w,
  divide by denominator row, LayerNorm (f32), MLP 48->256->128->32 in
  bf16 on PE, transpose out.
"""

import sys

if "/opt/trn_rl_repo" not in sys.path:
    sys.path.insert(0, "/opt/trn_rl_repo")

import numpy as np

N = 8192
D_IN = 128
D_H = 48
D_OUT = 32
N_CORES = 8
ROWS = N // N_CORES          # 1024 rows (i) per core
P = 128                      # partitions
NEG_SLOPE = 0.2
EPS = 1e-5
MOFF = 60.0                  # mask offset folded into slabG


def build_nc(num_cores=N_CORES, rows=ROWS, n=N, slab_bufs=4, mask_bufs=4,
             reps=1, exp_chunks=26, lrelu_mode="act",
             stages="dma,cmp,mm,epi"):
    import concourse.bass as bass
    import concourse.mybir as mybir
    import concourse.tile as tile
    from concourse import bacc
    from concourse.masks import make_identity
    from contextlib import ExitStack

    f32 = mybir.dt.float32
    f16 = mybir.dt.float16
    bf16 = mybir.dt.bfloat16
    AF = mybir.ActivationFunctionType
    OP = mybir.AluOpType

    n_chunk = n // P             # j-chunks
    n_half = rows // 512         # 512-wide i halves

    st = {}
    for tok in stages.split(","):
        name, _, mult = tok.partition(":")
        st[name] = int(mult) if mult else 1
    nc = bacc.Bacc("TRN2", target_bir_lowering=False, debug=False,
                   num_devices=num_cores)

    slab_d = nc.dram_tensor("slabg", [n, rows], bf16, kind="ExternalInput").ap()
    catwh_d = nc.dram_tensor("catwh", [n, P], bf16, kind="ExternalInput").ap()
    zwh_d = nc.dram_tensor("zwh", [n, 64], bf16, kind="ExternalInput").ap()
    whaug_d = nc.dram_tensor("whaug", [n, 64], bf16, kind="ExternalInput").ap()
    sP_d = nc.dram_tensor("sP", [P, n_chunk], f32, kind="ExternalInput").ap()
    sPn_d = nc.dram_tensor("sPn", [P, n_chunk], f32, kind="ExternalInput").ap()
    uw_d = nc.dram_tensor("uw", [2, rows], f32, kind="ExternalInput").ap()
    gamma_d = nc.dram_tensor("gamma", [1, D_H], f32, kind="ExternalInput").ap()
    beta_d = nc.dram_tensor("beta", [1, D_H], f32, kind="ExternalInput").ap()
    w1t_d = nc.dram_tensor("w1t", [D_H, 256], bf16, kind="ExternalInput").ap()
    b1_d = nc.dram_tensor("b1", [256, 1], f32, kind="ExternalInput").ap()
    w2t_d = nc.dram_tensor("w2t", [256, 128], bf16, kind="ExternalInput").ap()
    b2_d = nc.dram_tensor("b2", [128, 1], f32, kind="ExternalInput").ap()
    w3t_d = nc.dram_tensor("w3t", [128, D_OUT], bf16, kind="ExternalInput").ap()
    b3_d = nc.dram_tensor("b3", [D_OUT, 1], f32, kind="ExternalInput").ap()
    out_d = nc.dram_tensor("out", [rows, D_OUT], f32, kind="ExternalOutput").ap()

    with ExitStack() as ctx:
        tc = ctx.enter_context(tile.TileContext(nc))
        singles = ctx.enter_context(tc.tile_pool(name="singles", bufs=1))
        slabp = ctx.enter_context(tc.tile_pool(name="slabp", bufs=slab_bufs))
        maskp = ctx.enter_context(tc.tile_pool(name="maskp", bufs=mask_bufs))
        hp = ctx.enter_context(tc.tile_pool(name="hp", bufs=2))

        # ---- resident small tensors ----
        catwh_sb = singles.tile([P, n_chunk, P], bf16)
        nc.sync.dma_start(catwh_sb, catwh_d.rearrange("(c p) m -> p c m", p=P))
        zwh_sb = singles.tile([P, n_chunk, 64], bf16)
        nc.sync.dma_start(zwh_sb, zwh_d.rearrange("(c p) m -> p c m", p=P))
        sP_sb = singles.tile([P, n_chunk], f32)
        nc.sync.dma_start(sP_sb, sP_d)
        sPn_sb = singles.tile([P, n_chunk], f32)
        nc.sync.dma_start(sPn_sb, sPn_d)
        sP2_sb = singles.tile([P, n_chunk], f32)
        nc.scalar.activation(sP2_sb, sP_sb, AF.Copy, scale=NEG_SLOPE)
        whaug_sb = singles.tile([P, n_chunk, 64], bf16)
        nc.sync.dma_start(whaug_sb, whaug_d.rearrange("(c p) m -> p c m", p=P))
        # u replicated over partitions 0..63, w over all 128 (used at 64:113)
        u_rep = singles.tile([64, rows], f32)
        nc.sync.dma_start(u_rep, uw_d[0:1, :].partition_broadcast(64)
                          .rearrange("p one r -> p (one r)"))
        w_rep = singles.tile([P, rows], f32)
        nc.sync.dma_start(w_rep, uw_d[1:2, :].partition_broadcast(P)
                          .rearrange("p one r -> p (one r)"))
        gammaC = singles.tile([D_H, 1], f32)
        nc.sync.dma_start(gammaC, gamma_d.rearrange("one d -> d one"))
        betaC = singles.tile([D_H, 1], f32)
        nc.sync.dma_start(betaC, beta_d.rearrange("one d -> d one"))
        w1t_sb = singles.tile([D_H, 256], bf16)
        nc.sync.dma_start(w1t_sb, w1t_d)
        w2t_sb = singles.tile([P, 2, 128], bf16)
        nc.sync.dma_start(w2t_sb, w2t_d.rearrange("(m p) k -> p m k", p=P))
        w3t_sb = singles.tile([P, D_OUT], bf16)
        nc.sync.dma_start(w3t_sb, w3t_d)
        b1_sb = singles.tile([P, 2], f32)
        nc.sync.dma_start(b1_sb, b1_d.rearrange("(m p) one -> p (m one)", p=P))
        b2_sb = singles.tile([P, 1], f32)
        nc.sync.dma_start(b2_sb, b2_d)
        b3_sb = singles.tile([D_OUT, 1], f32)
        nc.sync.dma_start(b3_sb, b3_d)
        eps_sb = singles.tile([P, 1], f32)
        nc.vector.memset(eps_sb, EPS)
        ones48 = singles.tile([D_H, 1], bf16)
        nc.vector.memset(ones48, 1.0)
        ident = singles.tile([P, P], f32)
        make_identity(nc, ident)

        def bcast_sb(dst, src_row, parts, eng=None):
            src = bass.AP(tensor=src_row.tensor, offset=src_row.offset,
                          ap=[src_row.ap[0], [0, parts], src_row.ap[1]])
            dst3 = bass.AP(tensor=dst.tensor, offset=dst.offset,
                           ap=[dst.ap[0], [1, 1], dst.ap[1]])
            (eng or nc.sync).dma_start(dst3, src)

        slab_r = slab_d.rearrange("(c p) i -> p c i", p=P)

        # chunk classes: E (exp path on ACT, 1 matmul arm) / D (indicator
        # path on DVE, 2 matmul arms). Interleave E chunks for overlap.
        n_e = min(exp_chunks * n_chunk // 64, n_chunk)
        is_e = [False] * n_chunk
        cnt = 0
        for cc in range(n_chunk):
            if cnt < n_e and cc % 2 == 1:
                is_e[cc] = True
                cnt += 1
        for cc in range(n_chunk):
            if cnt < n_e and not is_e[cc]:
                is_e[cc] = True
                cnt += 1
        e_idx = [cc for cc in range(n_chunk) if is_e[cc]]
        d_idx = [cc for cc in range(n_chunk) if not is_e[cc]]
        for rep in range(reps):
          with tc.tile_pool(name=f"accp{rep}", bufs=2, space="PSUM") as accp:
            acc = [accp.tile([P, 512], f32, tag="acc", name=f"acc{h}")
                   for h in range(n_half)]
            accE = [accp.tile([64, 512], f32, tag="accE", name=f"accE{h}")
                    for h in range(n_half)] if e_idx else []
            for cc in range(n_chunk):
                slab = slabp.tile([P, rows], bf16, tag="slab")
                for _m in range(st.get("dma", 0)):
                    nc.sync.dma_start(slab, slab_r[:, cc, :])
                n_mm = st.get("mm", 0)
                if is_e[cc]:
                    te = maskp.tile([P, rows], bf16, tag="a1")
                    pe_ = maskp.tile([P, rows], bf16, tag="a0")
                    for _m in range(st.get("cmp", 0)):
                        if lrelu_mode == "act":
                            nc.scalar.activation(te, slab, AF.Prelu,
                                                 bias=sP_sb[:, cc:cc + 1],
                                                 alpha=NEG_SLOPE)
                            nc.scalar.activation(pe_, te, AF.Exp)
                        else:  # exp(leaky(x)) == max(exp(x), exp(0.2 x))
                            nc.scalar.activation(te, slab, AF.Exp,
                                                 scale=NEG_SLOPE,
                                                 bias=sP2_sb[:, cc:cc + 1])
                            nc.scalar.activation(pe_, slab, AF.Exp,
                                                 bias=sP_sb[:, cc:cc + 1])
                            nc.vector.tensor_tensor(pe_, pe_, te, OP.max)
                    for _m in range(n_mm):
                        for h in range(n_half):
                            sl = slice(h * 512, (h + 1) * 512)
                            nc.tensor.matmul(
                                accE[h][:, :],
                                lhsT=whaug_sb[:, cc, :],
                                rhs=pe_[:, sl],
                                start=(cc == e_idx[0] and _m == 0),
                                stop=(cc == e_idx[-1] and _m == n_mm - 1),
                                skip_group_check=True)
                    continue
                a1 = maskp.tile([P, rows], bf16, tag="a1")
                a0 = maskp.tile([P, rows], bf16, tag="a0")
                for _m in range(st.get("cmp", 0)):
                    nc.vector.tensor_scalar(a1, slab, sPn_sb[:, cc:cc + 1],
                                            None, OP.is_gt)
                    nc.vector.tensor_scalar(a0, slab, -(MOFF / 2), None,
                                            OP.is_gt)
                for _m in range(n_mm):
                    for h in range(n_half):
                        sl = slice(h * 512, (h + 1) * 512)
                        nc.tensor.matmul(
                            acc[h][:, :],
                            lhsT=catwh_sb[:, cc, :],
                            rhs=a1[:, sl],
                            start=(cc == d_idx[0] and _m == 0), stop=False,
                            skip_group_check=True)
                    for h in range(n_half):
                        sl = slice(h * 512, (h + 1) * 512)
                        nc.tensor.matmul(
                            acc[h][64:128, :],
                            lhsT=zwh_sb[:, cc, :],
                            rhs=a0[:, sl],
                            start=False,
                            stop=(cc == d_idx[-1] and _m == n_mm - 1),
                            skip_group_check=True)

            # ---- epilogue phase 1: combine, divide, LayerNorm ----
            hs = []
            do_epi = st.get("epi", 0) > 0 and st.get("mm", 0) > 0
            for h in range(n_half if do_epi else 0):
                sl = slice(h * 512, (h + 1) * 512)
                # combine: acc rows 0:49 hold vWh@A1 (u side), rows 64:113
                # hold zWh@(A0-A1) (w side). Weight each in place in PSUM,
                # bounce to SBUF, shift the w side down 64 partitions via
                # DMA, add, then divide by the denominator row.
                numT = hp.tile([49, 512], f32, tag="numT")
                if d_idx:
                    nc.vector.tensor_tensor(acc[h][64:113, :],
                                            acc[h][64:113, :],
                                            w_rep[64:113, sl], OP.mult)
                    nc.vector.tensor_tensor(acc[h][0:49, :], acc[h][0:49, :],
                                            u_rep[0:49, sl], OP.mult)
                    comb = hp.tile([P, 512], f32, tag="comb")
                    nc.scalar.activation(comb, acc[h][:, :], AF.Copy)
                    nb = hp.tile([49, 512], f32, tag="nb")
                    nc.scalar.dma_start(nb, comb[64:113, :])
                    nc.vector.tensor_tensor(numT, comb[0:49, :], nb, OP.add)
                    if e_idx:
                        nc.vector.tensor_tensor(numT, numT, accE[h][0:49, :],
                                                OP.add)
                else:
                    nc.vector.tensor_copy(numT, accE[h][0:49, :])
                den0 = hp.tile([1, 512], f32, tag="den0")
                nc.scalar.dma_start(den0, numT[48:49, :])
                rec = hp.tile([1, 512], f32, tag="rec")
                nc.vector.reciprocal_approx_fast(rec, den0)
                rbc = hp.tile([D_H, 512], f32, tag="rbc")
                bcast_sb(rbc, rec[0:1, :], D_H, eng=nc.scalar)
                hT = hp.tile([D_H, 512], f32, tag="hT", bufs=n_half)
                nc.vector.tensor_tensor(hT, numT[0:D_H, :], rbc, OP.mult)
                hT16 = hp.tile([D_H, 512], bf16, tag="hT16")
                nc.vector.tensor_copy(hT16, hT)
                sq = hp.tile([D_H, 512], bf16, tag="sq")
                nc.scalar.activation(sq, hT16, AF.Square)
                ssum = accp.tile([1, 512], f32, tag="ssum", name="ssum")
                nc.tensor.matmul(ssum, lhsT=ones48, rhs=hT16,
                                 start=True, stop=True)
                ssq = accp.tile([1, 512], f32, tag="ssq", name="ssq")
                nc.tensor.matmul(ssq, lhsT=ones48, rhs=sq,
                                 start=True, stop=True)
                mean = hp.tile([1, 512], f32, tag="mean")
                nc.scalar.activation(mean, ssum, AF.Copy, scale=1.0 / D_H)
                var = hp.tile([1, 512], f32, tag="var")
                nc.scalar.activation(var, ssq, AF.Copy, scale=1.0 / D_H,
                                     bias=EPS)
                msq = hp.tile([1, 512], f32, tag="msq")
                nc.vector.tensor_tensor(msq, mean, mean, OP.mult)
                nc.vector.tensor_tensor(var, var, msq, OP.subtract)
                std = hp.tile([1, 512], f32, tag="std")
                nc.scalar.activation(std, var, AF.Sqrt)
                rstd = hp.tile([1, 512], f32, tag="rstd")
                nc.vector.reciprocal_approx_fast(rstd, std)
                mbc = hp.tile([D_H, 512], f32, tag="mbc")
                bcast_sb(mbc, mean[0:1, :], D_H, eng=nc.scalar)
                sbc = hp.tile([D_H, 512], f32, tag="sbc")
                bcast_sb(sbc, rstd[0:1, :], D_H, eng=nc.scalar)
                nc.vector.tensor_tensor(hT, hT, mbc, OP.subtract)
                nc.vector.tensor_tensor(hT, hT, sbc, OP.mult)
                hTb = hp.tile([D_H, 512], bf16, tag="hTb", bufs=n_half)
                nc.vector.tensor_scalar(hTb, hT, gammaC, betaC,
                                        OP.mult, OP.add)
                hs.append(hTb)

          # ---- epilogue phase 2: MLP head in transposed layout (bf16) ----
          with tc.tile_pool(name=f"mlpp{rep}", bufs=1, space="PSUM") as mlpp:
            for h in range(n_half if do_epi else 0):
                h1 = hp.tile([P, 2, 512], bf16, tag="h1")
                for m in range(2):
                    m1 = mlpp.tile([P, 512], f32, tag="m1")
                    nc.tensor.matmul(m1, lhsT=w1t_sb[:, m * P:(m + 1) * P],
                                     rhs=hs[h], start=True, stop=True)
                    nc.scalar.activation(h1[:, m, :], m1, AF.Relu,
                                         bias=b1_sb[:, m:m + 1])
                m2 = mlpp.tile([P, 512], f32, tag="m2")
                for m in range(2):
                    nc.tensor.matmul(m2, lhsT=w2t_sb[:, m, :],
                                     rhs=h1[:, m, :],
                                     start=(m == 0), stop=(m == 1))
                h2 = hp.tile([P, 512], bf16, tag="h2")
                nc.scalar.activation(h2, m2, AF.Relu, bias=b2_sb)
                m3 = mlpp.tile([D_OUT, 512], f32, tag="m3")
                nc.tensor.matmul(m3, lhsT=w3t_sb, rhs=h2,
                                 start=True, stop=True)
                h3 = hp.tile([D_OUT, 512], f32, tag="h3")
                nc.scalar.activation(h3, m3, AF.Identity, bias=b3_sb)
                for k in range(4):
                    ko = h * 4 + k
                    m4 = mlpp.tile([P, D_OUT], f32, tag="m4")
                    nc.tensor.transpose(m4, h3[:, k * P:(k + 1) * P],
                                        ident[0:D_OUT, 0:D_OUT])
                    ob = hp.tile([P, D_OUT], f32, tag="ob")
                    nc.vector.tensor_copy(ob, m4)
                    nc.scalar.dma_start(out_d[ko * P:(ko + 1) * P, :], ob)

    nc.compile()
    return nc


def host_prep(x, adj, W_gat, a, gamma, beta, W1, b1, W2, b2, W3, b3,
              num_cores=N_CORES):
    import ml_dtypes

    bf16 = ml_dtypes.bfloat16
    n = x.shape[0]
    rows = n // num_cores
    n_chunk = n // P
    Wh = (x.astype(np.float32) @ W_gat.T.astype(np.float32))
    s = (Wh @ a.T.astype(np.float32)).ravel().astype(np.float32)
    assert np.abs(s).max() < MOFF / 2 - 0.1, "s out of slab-offset range"
    u = np.exp(s).astype(np.float32)          # exp(s)
    w = np.exp(NEG_SLOPE * s).astype(np.float32)
    # catwh: [vWh(48) v 0*15 | -zWh(48) -z 0*15]
    catwh = np.zeros((n, P), np.float32)
    catwh[:, 0:D_H] = u[:, None] * Wh
    catwh[:, D_H] = u
    catwh[:, 64:64 + D_H] = -(w[:, None] * Wh)
    catwh[:, 64 + D_H] = -w
    zwh = np.zeros((n, 64), np.float32)
    zwh[:, 0:D_H] = w[:, None] * Wh
    zwh[:, D_H] = w
    whaug = np.zeros((n, 64), np.float32)
    whaug[:, 0:D_H] = Wh
    whaug[:, D_H] = 1.0
    sP = np.ascontiguousarray(s.reshape(n_chunk, P).T)
    in_maps = []
    for c in range(num_cores):
        r = slice(c * rows, (c + 1) * rows)
        slabg = (s[r][None, :] +
                 MOFF * (adj[r].T.astype(np.float32) - 1.0)
                 ).astype(bf16)
        in_maps.append({
            "slabg": np.ascontiguousarray(slabg),
            "catwh": catwh.astype(bf16),
            "zwh": zwh.astype(bf16),
            "whaug": whaug.astype(bf16),
            "sP": sP,
            "sPn": np.ascontiguousarray(-sP),
            "uw": np.ascontiguousarray(np.stack([u[r], w[r]])),
            "gamma": np.ascontiguousarray(gamma[None, :]).astype(np.float32),
            "beta": np.ascontiguousarray(beta[None, :]).astype(np.float32),
            "w1t": np.ascontiguousarray(W1.T).astype(bf16),
            "b1": np.ascontiguousarray(b1[:, None]).astype(np.float32),
            "w2t": np.ascontiguousarray(W2.T).astype(bf16),
            "b2": np.ascontiguousarray(b2[:, None]).astype(np.float32),
            "w3t": np.ascontiguousarray(W3.T).astype(bf16),
            "b3": np.ascontiguousarray(b3[:, None]).astype(np.float32),
        })
    return in_maps


_NC_CACHE = {}


def kernel(x, adj, W_gat, a, gamma, beta, W1, b1, W2, b2, W3, b3,
           trace=False):
    from concourse.bass_utils import run_bass_kernel_spmd

    args = [np.asarray(t) for t in
            (x, adj, W_gat, a, gamma, beta, W1, b1, W2, b2, W3, b3)]
    in_maps = host_prep(*args)
    if "nc" not in _NC_CACHE:
        _NC_CACHE["nc"] = build_nc()
    nc = _NC_CACHE["nc"]
    res = run_bass_kernel_spmd(nc, in_maps, list(range(N_CORES)), trace=trace)
    out = np.concatenate([r["out"] for r in res.results], axis=0)
    if trace:
        kernel.last_results = res
    return out.astype(np.float32)


# revision 19
# speedup vs baseline: 1.1395x; 1.0983x over previous
"""Trainium2 Bass kernel for nn_MetaRL_LightGAT_BiACT (GAT + LayerNorm + MLP).

Strategy (8 NeuronCores, row-sharded, indicator-split formulation):

  exp(leaky_relu(s_i + s_j)) is exactly u_i*v_j when s_i+s_j > 0 and
  w_i*z_j otherwise, where u=exp(s), w=exp(0.2 s) (v=u, z=w over j).
  With c_ij = 1[s_i+s_j > 0] and A1 = adj*c, the GAT aggregation
  numerator (and denominator, via a ones column) becomes

     num_i = u_i * (A1 @ vWh)_i + w_i * ((adj @ zWh)_i - (A1 @ zWh)_i)

  i.e. two plain matmuls per j-chunk with 0/1 rhs masks -- no exp or
  leaky-relu over the N^2 data at all.

  Host precomputes a single pre-transposed fp16 slab
     slabG[j, i] = s_i + 4*(adj[i,j] - 1)
  from which BOTH masks fall out as one tensor_scalar each (4x DVE mode):
     A1 = (slabG + s_j) > 0        (adj=0 entries are < -2, never pass)
     A0 = slabG > -2               (recovers adj)

  Per j-chunk (128 j's x 1024 i's) on each core:
    DMA:  slabG chunk [128, 1024] fp16 (plain contiguous load)
    DVE:  A1 = ts(slabG add s_j, is_gt 0) -> bf16   (4x mode)
          A0 = ts(slabG is_gt -2)         -> bf16   (4x mode)
    PE:   accCat[0:128]  += [vWh | -zWh]_chunk^T @ A1   (bf16, 1 cyc/row)
          accCat[64:128] += zWh_chunk^T @ A0            (same PSUM bank;
                            accumulates zWh@(A0-A1) in rows 64..112)
  Epilogue: shift accCat[64:113] down via tiny DMA, combine with u/w,
  divide by denominator row, LayerNorm (f32), MLP 48->256->128->32 in
  bf16 on PE, transpose out.
"""

import sys

if "/opt/trn_rl_repo" not in sys.path:
    sys.path.insert(0, "/opt/trn_rl_repo")

import numpy as np

N = 8192
D_IN = 128
D_H = 48
D_OUT = 32
N_CORES = 8
ROWS = N // N_CORES          # 1024 rows (i) per core
P = 128                      # partitions
NEG_SLOPE = 0.2
EPS = 1e-5
MOFF = 60.0                  # mask offset folded into slabG


def build_nc(num_cores=N_CORES, rows=ROWS, n=N, slab_bufs=6, mask_bufs=6,
             reps=1, exp_chunks=28, lrelu_mode="act",
             stages="dma,cmp,mm,epi"):
    import concourse.bass as bass
    import concourse.mybir as mybir
    import concourse.tile as tile
    from concourse import bacc
    from concourse.masks import make_identity
    from contextlib import ExitStack

    f32 = mybir.dt.float32
    f16 = mybir.dt.float16
    bf16 = mybir.dt.bfloat16
    AF = mybir.ActivationFunctionType
    OP = mybir.AluOpType

    n_chunk = n // P             # j-chunks
    n_half = rows // 512         # 512-wide i halves

    st = {}
    for tok in stages.split(","):
        name, _, mult = tok.partition(":")
        st[name] = int(mult) if mult else 1
    nc = bacc.Bacc("TRN2", target_bir_lowering=False, debug=False,
                   num_devices=num_cores)

    slab_d = nc.dram_tensor("slabg", [n, rows], bf16, kind="ExternalInput").ap()
    catwh_d = nc.dram_tensor("catwh", [n, P], bf16, kind="ExternalInput").ap()
    zwh_d = nc.dram_tensor("zwh", [n, 64], bf16, kind="ExternalInput").ap()
    whaug_d = nc.dram_tensor("whaug", [n, 64], bf16, kind="ExternalInput").ap()
    sP_d = nc.dram_tensor("sP", [P, n_chunk], f32, kind="ExternalInput").ap()
    sPn_d = nc.dram_tensor("sPn", [P, n_chunk], f32, kind="ExternalInput").ap()
    uw_d = nc.dram_tensor("uw", [2, rows], f32, kind="ExternalInput").ap()
    gamma_d = nc.dram_tensor("gamma", [1, D_H], f32, kind="ExternalInput").ap()
    beta_d = nc.dram_tensor("beta", [1, D_H], f32, kind="ExternalInput").ap()
    w1t_d = nc.dram_tensor("w1t", [D_H, 256], bf16, kind="ExternalInput").ap()
    b1_d = nc.dram_tensor("b1", [256, 1], f32, kind="ExternalInput").ap()
    w2t_d = nc.dram_tensor("w2t", [256, 128], bf16, kind="ExternalInput").ap()
    b2_d = nc.dram_tensor("b2", [128, 1], f32, kind="ExternalInput").ap()
    w3t_d = nc.dram_tensor("w3t", [128, D_OUT], bf16, kind="ExternalInput").ap()
    b3_d = nc.dram_tensor("b3", [D_OUT, 1], f32, kind="ExternalInput").ap()
    out_d = nc.dram_tensor("out", [rows, D_OUT], f32, kind="ExternalOutput").ap()

    with ExitStack() as ctx:
        tc = ctx.enter_context(tile.TileContext(nc))
        singles = ctx.enter_context(tc.tile_pool(name="singles", bufs=1))
        slabp = ctx.enter_context(tc.tile_pool(name="slabp", bufs=slab_bufs))
        maskp = ctx.enter_context(tc.tile_pool(name="maskp", bufs=mask_bufs))
        hp = ctx.enter_context(tc.tile_pool(name="hp", bufs=2))

        # ---- resident small tensors ----
        catwh_sb = singles.tile([P, n_chunk, P], bf16)
        nc.sync.dma_start(catwh_sb, catwh_d.rearrange("(c p) m -> p c m", p=P))
        zwh_sb = singles.tile([P, n_chunk, 64], bf16)
        nc.sync.dma_start(zwh_sb, zwh_d.rearrange("(c p) m -> p c m", p=P))
        sP_sb = singles.tile([P, n_chunk], f32)
        nc.sync.dma_start(sP_sb, sP_d)
        sPn_sb = singles.tile([P, n_chunk], f32)
        nc.sync.dma_start(sPn_sb, sPn_d)
        sP2_sb = singles.tile([P, n_chunk], f32)
        nc.scalar.activation(sP2_sb, sP_sb, AF.Copy, scale=NEG_SLOPE)
        whaug_sb = singles.tile([P, n_chunk, 64], bf16)
        nc.sync.dma_start(whaug_sb, whaug_d.rearrange("(c p) m -> p c m", p=P))
        # u replicated over partitions 0..63, w over all 128 (used at 64:113)
        u_rep = singles.tile([64, rows], f32)
        nc.sync.dma_start(u_rep, uw_d[0:1, :].partition_broadcast(64)
                          .rearrange("p one r -> p (one r)"))
        w_rep = singles.tile([P, rows], f32)
        nc.sync.dma_start(w_rep, uw_d[1:2, :].partition_broadcast(P)
                          .rearrange("p one r -> p (one r)"))
        gammaC = singles.tile([D_H, 1], f32)
        nc.sync.dma_start(gammaC, gamma_d.rearrange("one d -> d one"))
        betaC = singles.tile([D_H, 1], f32)
        nc.sync.dma_start(betaC, beta_d.rearrange("one d -> d one"))
        w1t_sb = singles.tile([D_H, 256], bf16)
        nc.sync.dma_start(w1t_sb, w1t_d)
        w2t_sb = singles.tile([P, 2, 128], bf16)
        nc.sync.dma_start(w2t_sb, w2t_d.rearrange("(m p) k -> p m k", p=P))
        w3t_sb = singles.tile([P, D_OUT], bf16)
        nc.sync.dma_start(w3t_sb, w3t_d)
        b1_sb = singles.tile([P, 2], f32)
        nc.sync.dma_start(b1_sb, b1_d.rearrange("(m p) one -> p (m one)", p=P))
        b2_sb = singles.tile([P, 1], f32)
        nc.sync.dma_start(b2_sb, b2_d)
        b3_sb = singles.tile([D_OUT, 1], f32)
        nc.sync.dma_start(b3_sb, b3_d)
        eps_sb = singles.tile([P, 1], f32)
        nc.vector.memset(eps_sb, EPS)
        ones48 = singles.tile([D_H, 1], bf16)
        nc.vector.memset(ones48, 1.0)
        ident = singles.tile([P, P], f32)
        make_identity(nc, ident)

        def bcast_sb(dst, src_row, parts, eng=None):
            src = bass.AP(tensor=src_row.tensor, offset=src_row.offset,
                          ap=[src_row.ap[0], [0, parts], src_row.ap[1]])
            dst3 = bass.AP(tensor=dst.tensor, offset=dst.offset,
                           ap=[dst.ap[0], [1, 1], dst.ap[1]])
            (eng or nc.sync).dma_start(dst3, src)

        slab_r = slab_d.rearrange("(c p) i -> p c i", p=P)

        # chunk classes: E (exp path on ACT, 1 matmul arm) / D (indicator
        # path on DVE, 2 matmul arms). Interleave E chunks for overlap.
        n_e = min(exp_chunks * n_chunk // 64, n_chunk)
        is_e = [(cc * n_e // n_chunk) != ((cc + 1) * n_e // n_chunk)
                for cc in range(n_chunk)]
        e_idx = [cc for cc in range(n_chunk) if is_e[cc]]
        d_idx = [cc for cc in range(n_chunk) if not is_e[cc]]
        for rep in range(reps):
          with tc.tile_pool(name=f"accp{rep}", bufs=2, space="PSUM") as accp:
            acc = [accp.tile([P, 512], f32, tag="acc", name=f"acc{h}")
                   for h in range(n_half)]
            accE = [accp.tile([64, 512], f32, tag="accE", name=f"accE{h}")
                    for h in range(n_half)] if e_idx else []
            for cc in range(n_chunk):
                slab = slabp.tile([P, rows], bf16, tag="slab")
                for _m in range(st.get("dma", 0)):
                    nc.sync.dma_start(slab, slab_r[:, cc, :])
                n_mm = st.get("mm", 0)
                if is_e[cc]:
                    te = maskp.tile([P, rows], bf16, tag="a1")
                    pe_ = maskp.tile([P, rows], bf16, tag="a0")
                    for _m in range(st.get("cmp", 0)):
                        if lrelu_mode == "act":
                            nc.scalar.activation(te, slab, AF.Prelu,
                                                 bias=sP_sb[:, cc:cc + 1],
                                                 alpha=NEG_SLOPE)
                            nc.scalar.activation(pe_, te, AF.Exp)
                        else:  # exp(leaky(x)) == max(exp(x), exp(0.2 x))
                            nc.scalar.activation(te, slab, AF.Exp,
                                                 scale=NEG_SLOPE,
                                                 bias=sP2_sb[:, cc:cc + 1])
                            nc.scalar.activation(pe_, slab, AF.Exp,
                                                 bias=sP_sb[:, cc:cc + 1])
                            nc.vector.tensor_tensor(pe_, pe_, te, OP.max)
                    for _m in range(n_mm):
                        for h in range(n_half):
                            sl = slice(h * 512, (h + 1) * 512)
                            nc.tensor.matmul(
                                accE[h][:, :],
                                lhsT=whaug_sb[:, cc, :],
                                rhs=pe_[:, sl],
                                start=(cc == e_idx[0] and _m == 0),
                                stop=(cc == e_idx[-1] and _m == n_mm - 1),
                                skip_group_check=True)
                    continue
                a1 = maskp.tile([P, rows], bf16, tag="a1")
                a0 = maskp.tile([P, rows], bf16, tag="a0")
                for _m in range(st.get("cmp", 0)):
                    nc.vector.tensor_scalar(a1, slab, sPn_sb[:, cc:cc + 1],
                                            None, OP.is_gt)
                    nc.vector.tensor_scalar(a0, slab, -(MOFF / 2), None,
                                            OP.is_gt)
                for _m in range(n_mm):
                    for h in range(n_half):
                        sl = slice(h * 512, (h + 1) * 512)
                        nc.tensor.matmul(
                            acc[h][:, :],
                            lhsT=catwh_sb[:, cc, :],
                            rhs=a1[:, sl],
                            start=(cc == d_idx[0] and _m == 0), stop=False,
                            skip_group_check=True)
                    for h in range(n_half):
                        sl = slice(h * 512, (h + 1) * 512)
                        nc.tensor.matmul(
                            acc[h][64:128, :],
                            lhsT=zwh_sb[:, cc, :],
                            rhs=a0[:, sl],
                            start=False,
                            stop=(cc == d_idx[-1] and _m == n_mm - 1),
                            skip_group_check=True)

            # ---- epilogue phase 1: combine, divide, LayerNorm ----
            hs = []
            do_epi = st.get("epi", 0) > 0 and st.get("mm", 0) > 0
            for h in range(n_half if do_epi else 0):
                sl = slice(h * 512, (h + 1) * 512)
                # combine: acc rows 0:49 hold vWh@A1 (u side), rows 64:113
                # hold zWh@(A0-A1) (w side). Weight each in place in PSUM,
                # bounce to SBUF, shift the w side down 64 partitions via
                # DMA, add, then divide by the denominator row.
                numT = hp.tile([49, 512], f32, tag="numT")
                if d_idx:
                    nc.vector.tensor_tensor(acc[h][64:113, :],
                                            acc[h][64:113, :],
                                            w_rep[64:113, sl], OP.mult)
                    nc.vector.tensor_tensor(acc[h][0:49, :], acc[h][0:49, :],
                                            u_rep[0:49, sl], OP.mult)
                    comb = hp.tile([P, 512], f32, tag="comb")
                    nc.vector.tensor_copy(comb, acc[h][:, :])
                    nb = hp.tile([49, 512], f32, tag="nb")
                    nc.gpsimd.dma_start(nb, comb[64:113, :])
                    nc.vector.tensor_tensor(numT, comb[0:49, :], nb, OP.add)
                    if e_idx:
                        nc.vector.tensor_tensor(numT, numT, accE[h][0:49, :],
                                                OP.add)
                else:
                    nc.vector.tensor_copy(numT, accE[h][0:49, :])
                den0 = hp.tile([1, 512], f32, tag="den0")
                nc.gpsimd.dma_start(den0, numT[48:49, :])
                rec = hp.tile([1, 512], f32, tag="rec")
                nc.vector.reciprocal_approx_fast(rec, den0)
                rbc = hp.tile([D_H, 512], f32, tag="rbc")
                bcast_sb(rbc, rec[0:1, :], D_H, eng=nc.gpsimd)
                hT = hp.tile([D_H, 512], f32, tag="hT", bufs=n_half)
                nc.vector.tensor_tensor(hT, numT[0:D_H, :], rbc, OP.mult)
                hT16 = hp.tile([D_H, 512], bf16, tag="hT16")
                nc.vector.tensor_copy(hT16, hT)
                sq = hp.tile([D_H, 512], bf16, tag="sq")
                nc.scalar.activation(sq, hT16, AF.Square)
                ssum = accp.tile([1, 512], f32, tag="ssum", name="ssum")
                nc.tensor.matmul(ssum, lhsT=ones48, rhs=hT16,
                                 start=True, stop=True)
                ssq = accp.tile([1, 512], f32, tag="ssq", name="ssq")
                nc.tensor.matmul(ssq, lhsT=ones48, rhs=sq,
                                 start=True, stop=True)
                mean = hp.tile([1, 512], f32, tag="mean")
                nc.vector.tensor_scalar(mean, ssum, 1.0 / D_H, None, OP.mult)
                var = hp.tile([1, 512], f32, tag="var")
                nc.vector.tensor_scalar(var, ssq, 1.0 / D_H, EPS,
                                        OP.mult, OP.add)
                msq = hp.tile([1, 512], f32, tag="msq")
                nc.vector.tensor_tensor(msq, mean, mean, OP.mult)
                nc.vector.tensor_tensor(var, var, msq, OP.subtract)
                std = hp.tile([1, 512], f32, tag="std")
                nc.scalar.activation(std, var, AF.Sqrt)
                rstd = hp.tile([1, 512], f32, tag="rstd")
                nc.vector.reciprocal_approx_fast(rstd, std)
                mbc = hp.tile([D_H, 512], f32, tag="mbc")
                bcast_sb(mbc, mean[0:1, :], D_H, eng=nc.gpsimd)
                sbc = hp.tile([D_H, 512], f32, tag="sbc")
                bcast_sb(sbc, rstd[0:1, :], D_H, eng=nc.gpsimd)
                nc.vector.tensor_tensor(hT, hT, mbc, OP.subtract)
                nc.vector.tensor_tensor(hT, hT, sbc, OP.mult)
                hTb = hp.tile([D_H, 512], bf16, tag="hTb", bufs=n_half)
                nc.vector.tensor_scalar(hTb, hT, gammaC, betaC,
                                        OP.mult, OP.add)
                hs.append(hTb)

          # ---- epilogue phase 2: MLP head in transposed layout (bf16) ----
          with tc.tile_pool(name=f"mlpp{rep}", bufs=1, space="PSUM") as mlpp:
            for h in range(n_half if do_epi else 0):
                h1 = hp.tile([P, 2, 512], bf16, tag="h1")
                for m in range(2):
                    m1 = mlpp.tile([P, 512], f32, tag="m1")
                    nc.tensor.matmul(m1, lhsT=w1t_sb[:, m * P:(m + 1) * P],
                                     rhs=hs[h], start=True, stop=True)
                    nc.scalar.activation(h1[:, m, :], m1, AF.Relu,
                                         bias=b1_sb[:, m:m + 1])
                m2 = mlpp.tile([P, 512], f32, tag="m2")
                for m in range(2):
                    nc.tensor.matmul(m2, lhsT=w2t_sb[:, m, :],
                                     rhs=h1[:, m, :],
                                     start=(m == 0), stop=(m == 1))
                h2 = hp.tile([P, 512], bf16, tag="h2")
                nc.scalar.activation(h2, m2, AF.Relu, bias=b2_sb)
                m3 = mlpp.tile([D_OUT, 512], f32, tag="m3")
                nc.tensor.matmul(m3, lhsT=w3t_sb, rhs=h2,
                                 start=True, stop=True)
                h3 = hp.tile([D_OUT, 512], f32, tag="h3")
                nc.scalar.activation(h3, m3, AF.Identity, bias=b3_sb)
                for k in range(4):
                    ko = h * 4 + k
                    m4 = mlpp.tile([P, D_OUT], f32, tag="m4")
                    nc.tensor.transpose(m4, h3[:, k * P:(k + 1) * P],
                                        ident[0:D_OUT, 0:D_OUT])
                    ob = hp.tile([P, D_OUT], f32, tag="ob")
                    nc.vector.tensor_copy(ob, m4)
                    nc.gpsimd.dma_start(out_d[ko * P:(ko + 1) * P, :], ob)

    nc.compile()
    return nc


def host_prep(x, adj, W_gat, a, gamma, beta, W1, b1, W2, b2, W3, b3,
              num_cores=N_CORES):
    import ml_dtypes

    bf16 = ml_dtypes.bfloat16
    n = x.shape[0]
    rows = n // num_cores
    n_chunk = n // P
    Wh = (x.astype(np.float32) @ W_gat.T.astype(np.float32))
    s = (Wh @ a.T.astype(np.float32)).ravel().astype(np.float32)
    assert np.abs(s).max() < MOFF / 2 - 0.1, "s out of slab-offset range"
    u = np.exp(s).astype(np.float32)          # exp(s)
    w = np.exp(NEG_SLOPE * s).astype(np.float32)
    # catwh: [vWh(48) v 0*15 | -zWh(48) -z 0*15]
    catwh = np.zeros((n, P), np.float32)
    catwh[:, 0:D_H] = u[:, None] * Wh
    catwh[:, D_H] = u
    catwh[:, 64:64 + D_H] = -(w[:, None] * Wh)
    catwh[:, 64 + D_H] = -w
    zwh = np.zeros((n, 64), np.float32)
    zwh[:, 0:D_H] = w[:, None] * Wh
    zwh[:, D_H] = w
    whaug = np.zeros((n, 64), np.float32)
    whaug[:, 0:D_H] = Wh
    whaug[:, D_H] = 1.0
    sP = np.ascontiguousarray(s.reshape(n_chunk, P).T)
    in_maps = []
    for c in range(num_cores):
        r = slice(c * rows, (c + 1) * rows)
        slabg = (s[r][None, :] +
                 MOFF * (adj[r].T.astype(np.float32) - 1.0)
                 ).astype(bf16)
        in_maps.append({
            "slabg": np.ascontiguousarray(slabg),
            "catwh": catwh.astype(bf16),
            "zwh": zwh.astype(bf16),
            "whaug": whaug.astype(bf16),
            "sP": sP,
            "sPn": np.ascontiguousarray(-sP),
            "uw": np.ascontiguousarray(np.stack([u[r], w[r]])),
            "gamma": np.ascontiguousarray(gamma[None, :]).astype(np.float32),
            "beta": np.ascontiguousarray(beta[None, :]).astype(np.float32),
            "w1t": np.ascontiguousarray(W1.T).astype(bf16),
            "b1": np.ascontiguousarray(b1[:, None]).astype(np.float32),
            "w2t": np.ascontiguousarray(W2.T).astype(bf16),
            "b2": np.ascontiguousarray(b2[:, None]).astype(np.float32),
            "w3t": np.ascontiguousarray(W3.T).astype(bf16),
            "b3": np.ascontiguousarray(b3[:, None]).astype(np.float32),
        })
    return in_maps


_NC_CACHE = {}


def kernel(x, adj, W_gat, a, gamma, beta, W1, b1, W2, b2, W3, b3,
           trace=False):
    from concourse.bass_utils import run_bass_kernel_spmd

    args = [np.asarray(t) for t in
            (x, adj, W_gat, a, gamma, beta, W1, b1, W2, b2, W3, b3)]
    in_maps = host_prep(*args)
    if "nc" not in _NC_CACHE:
        _NC_CACHE["nc"] = build_nc()
    nc = _NC_CACHE["nc"]
    res = run_bass_kernel_spmd(nc, in_maps, list(range(N_CORES)), trace=trace)
    out = np.concatenate([r["out"] for r in res.results], axis=0)
    if trace:
        kernel.last_results = res
    return out.astype(np.float32)


# revision 25
# speedup vs baseline: 1.1765x; 1.0325x over previous
"""Trainium2 Bass kernel for nn_MetaRL_LightGAT_BiACT (GAT + LayerNorm + MLP).

Strategy (8 NeuronCores, row-sharded, indicator-split formulation):

  exp(leaky_relu(s_i + s_j)) is exactly u_i*v_j when s_i+s_j > 0 and
  w_i*z_j otherwise, where u=exp(s), w=exp(0.2 s) (v=u, z=w over j).
  With c_ij = 1[s_i+s_j > 0] and A1 = adj*c, the GAT aggregation
  numerator (and denominator, via a ones column) becomes

     num_i = u_i * (A1 @ vWh)_i + w_i * ((adj @ zWh)_i - (A1 @ zWh)_i)

  i.e. two plain matmuls per j-chunk with 0/1 rhs masks -- no exp or
  leaky-relu over the N^2 data at all.

  Host precomputes a single pre-transposed fp16 slab
     slabG[j, i] = s_i + 4*(adj[i,j] - 1)
  from which BOTH masks fall out as one tensor_scalar each (4x DVE mode):
     A1 = (slabG + s_j) > 0        (adj=0 entries are < -2, never pass)
     A0 = slabG > -2               (recovers adj)

  Per j-chunk (128 j's x 1024 i's) on each core:
    DMA:  slabG chunk [128, 1024] fp16 (plain contiguous load)
    DVE:  A1 = ts(slabG add s_j, is_gt 0) -> bf16   (4x mode)
          A0 = ts(slabG is_gt -2)         -> bf16   (4x mode)
    PE:   accCat[0:128]  += [vWh | -zWh]_chunk^T @ A1   (bf16, 1 cyc/row)
          accCat[64:128] += zWh_chunk^T @ A0            (same PSUM bank;
                            accumulates zWh@(A0-A1) in rows 64..112)
  Epilogue: shift accCat[64:113] down via tiny DMA, combine with u/w,
  divide by denominator row, LayerNorm (f32), MLP 48->256->128->32 in
  bf16 on PE, transpose out.
"""

import sys

if "/opt/trn_rl_repo" not in sys.path:
    sys.path.insert(0, "/opt/trn_rl_repo")

import numpy as np

N = 8192
D_IN = 128
D_H = 48
D_OUT = 32
N_CORES = 8
ROWS = N // N_CORES          # 1024 rows (i) per core
P = 128                      # partitions
NEG_SLOPE = 0.2
EPS = 1e-5
MOFF = 60.0                  # mask offset folded into slabG


def build_nc(num_cores=N_CORES, rows=ROWS, n=N, slab_bufs=6, mask_bufs=6,
             reps=1, exp_chunks=28, lrelu_mode="act",
             stages="dma,cmp,mm,epi"):
    import concourse.bass as bass
    import concourse.mybir as mybir
    import concourse.tile as tile
    from concourse import bacc
    from concourse.masks import make_identity
    from contextlib import ExitStack

    f32 = mybir.dt.float32
    f16 = mybir.dt.float16
    bf16 = mybir.dt.bfloat16
    AF = mybir.ActivationFunctionType
    OP = mybir.AluOpType

    n_chunk = n // P             # j-chunks
    n_half = rows // 512         # 512-wide i halves

    st = {}
    for tok in stages.split(","):
        name, _, mult = tok.partition(":")
        st[name] = int(mult) if mult else 1
    nc = bacc.Bacc("TRN2", target_bir_lowering=False, debug=False,
                   num_devices=num_cores)

    slab_d = nc.dram_tensor("slabg", [n, rows], bf16, kind="ExternalInput").ap()
    catwh_d = nc.dram_tensor("catwh", [n, P], bf16, kind="ExternalInput").ap()
    zwh_d = nc.dram_tensor("zwh", [n, 64], bf16, kind="ExternalInput").ap()
    whaug_d = nc.dram_tensor("whaug", [n, 64], bf16, kind="ExternalInput").ap()
    sP_d = nc.dram_tensor("sP", [P, n_chunk], f32, kind="ExternalInput").ap()
    sPn_d = nc.dram_tensor("sPn", [P, n_chunk], f32, kind="ExternalInput").ap()
    uw_d = nc.dram_tensor("uw", [2, rows], f32, kind="ExternalInput").ap()
    dsel_d = nc.dram_tensor("dsel", [P, 1], f32, kind="ExternalInput").ap()
    gamma_d = nc.dram_tensor("gamma", [1, D_H], f32, kind="ExternalInput").ap()
    beta_d = nc.dram_tensor("beta", [1, D_H], f32, kind="ExternalInput").ap()
    w1t_d = nc.dram_tensor("w1t", [D_H, 256], bf16, kind="ExternalInput").ap()
    b1_d = nc.dram_tensor("b1", [256, 1], f32, kind="ExternalInput").ap()
    w2t_d = nc.dram_tensor("w2t", [256, 128], bf16, kind="ExternalInput").ap()
    b2_d = nc.dram_tensor("b2", [128, 1], f32, kind="ExternalInput").ap()
    w3t_d = nc.dram_tensor("w3t", [128, D_OUT], bf16, kind="ExternalInput").ap()
    b3_d = nc.dram_tensor("b3", [D_OUT, 1], f32, kind="ExternalInput").ap()
    out_d = nc.dram_tensor("out", [rows, D_OUT], f32, kind="ExternalOutput").ap()

    with ExitStack() as ctx:
        tc = ctx.enter_context(tile.TileContext(nc))
        singles = ctx.enter_context(tc.tile_pool(name="singles", bufs=1))
        slabp = ctx.enter_context(tc.tile_pool(name="slabp", bufs=slab_bufs))
        maskp = ctx.enter_context(tc.tile_pool(name="maskp", bufs=mask_bufs))
        hp = ctx.enter_context(tc.tile_pool(name="hp", bufs=2))

        # ---- resident small tensors ----
        catwh_sb = singles.tile([P, n_chunk, P], bf16)
        nc.sync.dma_start(catwh_sb, catwh_d.rearrange("(c p) m -> p c m", p=P))
        zwh_sb = singles.tile([P, n_chunk, 64], bf16)
        nc.sync.dma_start(zwh_sb, zwh_d.rearrange("(c p) m -> p c m", p=P))
        sP_sb = singles.tile([P, n_chunk], f32)
        nc.sync.dma_start(sP_sb, sP_d)
        sPn_sb = singles.tile([P, n_chunk], f32)
        nc.sync.dma_start(sPn_sb, sPn_d)
        sP2_sb = singles.tile([P, n_chunk], f32)
        nc.scalar.activation(sP2_sb, sP_sb, AF.Copy, scale=NEG_SLOPE)
        whaug_sb = singles.tile([P, n_chunk, 64], bf16)
        nc.sync.dma_start(whaug_sb, whaug_d.rearrange("(c p) m -> p c m", p=P))
        # u replicated over partitions 0..63, w over all 128 (used at 64:113)
        u_rep = singles.tile([64, rows], f32)
        nc.sync.dma_start(u_rep, uw_d[0:1, :].partition_broadcast(64)
                          .rearrange("p one r -> p (one r)"))
        w_rep = singles.tile([P, rows], f32)
        nc.sync.dma_start(w_rep, uw_d[1:2, :].partition_broadcast(P)
                          .rearrange("p one r -> p (one r)"))
        gammaC = singles.tile([D_H, 1], f32)
        nc.sync.dma_start(gammaC, gamma_d.rearrange("one d -> d one"))
        betaC = singles.tile([D_H, 1], f32)
        nc.sync.dma_start(betaC, beta_d.rearrange("one d -> d one"))
        w1t_sb = singles.tile([D_H, 256], bf16)
        nc.sync.dma_start(w1t_sb, w1t_d)
        w2t_sb = singles.tile([P, 2, 128], bf16)
        nc.sync.dma_start(w2t_sb, w2t_d.rearrange("(m p) k -> p m k", p=P))
        w3t_sb = singles.tile([P, D_OUT], bf16)
        nc.sync.dma_start(w3t_sb, w3t_d)
        b1_sb = singles.tile([P, 2], f32)
        nc.sync.dma_start(b1_sb, b1_d.rearrange("(m p) one -> p (m one)", p=P))
        b2_sb = singles.tile([P, 1], f32)
        nc.sync.dma_start(b2_sb, b2_d)
        b3_sb = singles.tile([D_OUT, 1], f32)
        nc.sync.dma_start(b3_sb, b3_d)
        eps_sb = singles.tile([P, 1], f32)
        nc.vector.memset(eps_sb, EPS)
        dsel_sb = singles.tile([P, 1], f32)
        nc.sync.dma_start(dsel_sb, dsel_d)
        ones1x48 = singles.tile([1, D_H], f32)
        nc.vector.memset(ones1x48, 1.0)
        ones48 = singles.tile([D_H, 1], bf16)
        nc.vector.memset(ones48, 1.0)
        ident = singles.tile([P, P], f32)
        make_identity(nc, ident)

        def bcast_sb(dst, src_row, parts, eng=None):
            src = bass.AP(tensor=src_row.tensor, offset=src_row.offset,
                          ap=[src_row.ap[0], [0, parts], src_row.ap[1]])
            dst3 = bass.AP(tensor=dst.tensor, offset=dst.offset,
                           ap=[dst.ap[0], [1, 1], dst.ap[1]])
            (eng or nc.sync).dma_start(dst3, src)

        slab_r = slab_d.rearrange("(c p) i -> p c i", p=P)

        # chunk classes: E (exp path on ACT, 1 matmul arm) / D (indicator
        # path on DVE, 2 matmul arms). Interleave E chunks for overlap.
        n_e = min(exp_chunks * n_chunk // 64, n_chunk)
        is_e = [(cc * n_e // n_chunk) != ((cc + 1) * n_e // n_chunk)
                for cc in range(n_chunk)]
        e_idx = [cc for cc in range(n_chunk) if is_e[cc]]
        d_idx = [cc for cc in range(n_chunk) if not is_e[cc]]
        for rep in range(reps):
          with tc.tile_pool(name=f"accp{rep}", bufs=2, space="PSUM") as accp:
            acc = [accp.tile([P, 512], f32, tag="acc", name=f"acc{h}")
                   for h in range(n_half)]
            accE = [accp.tile([64, 512], f32, tag="accE", name=f"accE{h}")
                    for h in range(n_half)]
            for cc in range(n_chunk):
                slab = slabp.tile([P, rows], bf16, tag="slab")
                for _m in range(st.get("dma", 0)):
                    nc.sync.dma_start(slab, slab_r[:, cc, :])
                n_mm = st.get("mm", 0)
                if is_e[cc]:
                    te = maskp.tile([P, rows], bf16, tag="a1")
                    pe_ = maskp.tile([P, rows], bf16, tag="a0")
                    for _m in range(st.get("cmp", 0)):
                        if lrelu_mode == "act":
                            nc.scalar.activation(te, slab, AF.Prelu,
                                                 bias=sP_sb[:, cc:cc + 1],
                                                 alpha=NEG_SLOPE)
                            nc.scalar.activation(pe_, te, AF.Exp)
                        else:  # exp(leaky(x)) == max(exp(x), exp(0.2 x))
                            nc.scalar.activation(te, slab, AF.Exp,
                                                 scale=NEG_SLOPE,
                                                 bias=sP2_sb[:, cc:cc + 1])
                            nc.scalar.activation(pe_, slab, AF.Exp,
                                                 bias=sP_sb[:, cc:cc + 1])
                            nc.vector.tensor_tensor(pe_, pe_, te, OP.max)
                    for _m in range(n_mm):
                        for h in range(n_half):
                            sl = slice(h * 512, (h + 1) * 512)
                            nc.tensor.matmul(
                                accE[h][:, :],
                                lhsT=whaug_sb[:, cc, :],
                                rhs=pe_[:, sl],
                                start=(cc == e_idx[0] and _m == 0),
                                stop=(cc == e_idx[-1] and _m == n_mm - 1
                                      and not d_idx),
                                skip_group_check=True)
                    continue
                a1 = maskp.tile([P, rows], bf16, tag="a1")
                a0 = maskp.tile([P, rows], bf16, tag="a0")
                for _m in range(st.get("cmp", 0)):
                    nc.vector.tensor_scalar(a1, slab, sPn_sb[:, cc:cc + 1],
                                            None, OP.is_gt)
                    nc.vector.tensor_scalar(a0, slab, -(MOFF / 2), None,
                                            OP.is_gt)
                for _m in range(n_mm):
                    for h in range(n_half):
                        sl = slice(h * 512, (h + 1) * 512)
                        nc.tensor.matmul(
                            acc[h][:, :],
                            lhsT=catwh_sb[:, cc, :],
                            rhs=a1[:, sl],
                            start=(cc == d_idx[0] and _m == 0), stop=False,
                            skip_group_check=True)
                    for h in range(n_half):
                        sl = slice(h * 512, (h + 1) * 512)
                        nc.tensor.matmul(
                            acc[h][64:128, :],
                            lhsT=zwh_sb[:, cc, :],
                            rhs=a0[:, sl],
                            start=False,
                            stop=(cc == d_idx[-1] and _m == n_mm - 1),
                            skip_group_check=True)

            # ---- epilogue phase 1: combine, divide, LayerNorm ----
            hs = []
            do_epi = st.get("epi", 0) > 0 and st.get("mm", 0) > 0
            for h in range(n_half if do_epi else 0):
                sl = slice(h * 512, (h + 1) * 512)
                # combine: acc rows 0:49 hold vWh@A1 (u side), rows 64:113
                # hold zWh@(A0-A1) (w side). Weight each in place in PSUM,
                # bounce to SBUF, shift the w side down 64 partitions via
                # DMA, add, then divide by the denominator row.
                numT = hp.tile([49, 512], f32, tag="numT")
                if d_idx:
                    nc.vector.tensor_tensor(acc[h][64:113, :],
                                            acc[h][64:113, :],
                                            w_rep[64:113, sl], OP.mult)
                    nc.vector.tensor_tensor(acc[h][0:49, :], acc[h][0:49, :],
                                            u_rep[0:49, sl], OP.mult)
                    comb = hp.tile([P, 512], f32, tag="comb")
                    nc.vector.tensor_copy(comb, acc[h][:, :])
                    # fold the 64->0 partition shift into the accE group:
                    # accE[0:49] += comb[64:113] via identity-slice matmul
                    nc.tensor.matmul(accE[h][0:49, :],
                                     lhsT=ident[:, 64:113], rhs=comb,
                                     start=(not e_idx), stop=True,
                                     skip_group_check=True)
                    nc.vector.tensor_tensor(numT, comb[0:49, :],
                                            accE[h][0:49, :], OP.add)
                else:
                    nc.vector.tensor_copy(numT, accE[h][0:49, :])
                den0 = hp.tile([1, 512], f32, tag="den0")
                nc.gpsimd.dma_start(den0, numT[48:49, :])
                rec = hp.tile([1, 512], f32, tag="rec")
                nc.vector.reciprocal_approx_fast(rec, den0)
                rbc = hp.tile([D_H, 512], f32, tag="rbc")
                bcast_sb(rbc, rec[0:1, :], D_H, eng=nc.gpsimd)
                hT = hp.tile([D_H, 512], bf16, tag="hT", bufs=n_half)
                nc.vector.tensor_tensor(hT, numT[0:D_H, :], rbc, OP.mult)
                sq = hp.tile([D_H, 512], bf16, tag="sq")
                nc.scalar.activation(sq, hT, AF.Square)
                ssum = accp.tile([1, 512], f32, tag="ssum", name="ssum")
                nc.tensor.matmul(ssum, lhsT=ones48, rhs=hT,
                                 start=True, stop=True)
                ssq = accp.tile([1, 512], f32, tag="ssq", name="ssq")
                nc.tensor.matmul(ssq, lhsT=ones48, rhs=sq,
                                 start=True, stop=True)
                mean = hp.tile([1, 512], f32, tag="mean")
                nc.vector.tensor_scalar(mean, ssum, 1.0 / D_H, None, OP.mult)
                var = hp.tile([1, 512], f32, tag="var")
                nc.vector.tensor_scalar(var, ssq, 1.0 / D_H, EPS,
                                        OP.mult, OP.add)
                msq = hp.tile([1, 512], f32, tag="msq")
                nc.vector.tensor_tensor(msq, mean, mean, OP.mult)
                nc.vector.tensor_tensor(var, var, msq, OP.subtract)
                std = hp.tile([1, 512], f32, tag="std")
                nc.scalar.activation(std, var, AF.Sqrt)
                rstd = hp.tile([1, 512], f32, tag="rstd")
                nc.vector.reciprocal_approx_fast(rstd, std)
                mbc = hp.tile([D_H, 512], f32, tag="mbc")
                bcast_sb(mbc, mean[0:1, :], D_H, eng=nc.gpsimd)
                sbc = hp.tile([D_H, 512], f32, tag="sbc")
                bcast_sb(sbc, rstd[0:1, :], D_H, eng=nc.gpsimd)
                nc.vector.tensor_tensor(hT, hT, mbc, OP.subtract)
                nc.vector.tensor_tensor(hT, hT, sbc, OP.mult)
                hTb = hp.tile([D_H, 512], bf16, tag="hTb", bufs=n_half)
                nc.vector.tensor_scalar(hTb, hT, gammaC, betaC,
                                        OP.mult, OP.add)
                hs.append(hTb)

          # ---- epilogue phase 2: MLP head in transposed layout (bf16) ----
          with tc.tile_pool(name=f"mlpp{rep}", bufs=1, space="PSUM") as mlpp:
            for h in range(n_half if do_epi else 0):
                h1 = hp.tile([P, 2, 512], bf16, tag="h1")
                for m in range(2):
                    m1 = mlpp.tile([P, 512], f32, tag="m1")
                    nc.tensor.matmul(m1, lhsT=w1t_sb[:, m * P:(m + 1) * P],
                                     rhs=hs[h], start=True, stop=True)
                    nc.scalar.activation(h1[:, m, :], m1, AF.Relu,
                                         bias=b1_sb[:, m:m + 1])
                m2 = mlpp.tile([P, 512], f32, tag="m2")
                for m in range(2):
                    nc.tensor.matmul(m2, lhsT=w2t_sb[:, m, :],
                                     rhs=h1[:, m, :],
                                     start=(m == 0), stop=(m == 1))
                h2 = hp.tile([P, 512], bf16, tag="h2")
                nc.scalar.activation(h2, m2, AF.Relu, bias=b2_sb)
                m3 = mlpp.tile([D_OUT, 512], f32, tag="m3")
                nc.tensor.matmul(m3, lhsT=w3t_sb, rhs=h2,
                                 start=True, stop=True)
                h3 = hp.tile([D_OUT, 512], f32, tag="h3")
                nc.scalar.activation(h3, m3, AF.Identity, bias=b3_sb)
                for k in range(4):
                    ko = h * 4 + k
                    m4 = mlpp.tile([P, D_OUT], f32, tag="m4")
                    nc.tensor.transpose(m4, h3[:, k * P:(k + 1) * P],
                                        ident[0:D_OUT, 0:D_OUT])
                    ob = hp.tile([P, D_OUT], f32, tag="ob")
                    nc.vector.tensor_copy(ob, m4)
                    nc.gpsimd.dma_start(out_d[ko * P:(ko + 1) * P, :], ob)

    nc.compile()
    return nc


def host_prep(x, adj, W_gat, a, gamma, beta, W1, b1, W2, b2, W3, b3,
              num_cores=N_CORES):
    import ml_dtypes

    bf16 = ml_dtypes.bfloat16
    n = x.shape[0]
    rows = n // num_cores
    n_chunk = n // P
    Wh = (x.astype(np.float32) @ W_gat.T.astype(np.float32))
    s = (Wh @ a.T.astype(np.float32)).ravel().astype(np.float32)
    assert np.abs(s).max() < MOFF / 2 - 0.1, "s out of slab-offset range"
    u = np.exp(s).astype(np.float32)          # exp(s)
    w = np.exp(NEG_SLOPE * s).astype(np.float32)
    # catwh: [vWh(48) v 0*15 | -zWh(48) -z 0*15]
    catwh = np.zeros((n, P), np.float32)
    catwh[:, 0:D_H] = u[:, None] * Wh
    catwh[:, D_H] = u
    catwh[:, 64:64 + D_H] = -(w[:, None] * Wh)
    catwh[:, 64 + D_H] = -w
    zwh = np.zeros((n, 64), np.float32)
    zwh[:, 0:D_H] = w[:, None] * Wh
    zwh[:, D_H] = w
    whaug = np.zeros((n, 64), np.float32)
    whaug[:, 0:D_H] = Wh
    whaug[:, D_H] = 1.0
    sP = np.ascontiguousarray(s.reshape(n_chunk, P).T)
    dsel = np.zeros((P, 1), np.float32)
    dsel[D_H, 0] = 1.0
    dsel[64 + D_H, 0] = 1.0
    in_maps = []
    for c in range(num_cores):
        r = slice(c * rows, (c + 1) * rows)
        slabg = (s[r][None, :] +
                 MOFF * (adj[r].T.astype(np.float32) - 1.0)
                 ).astype(bf16)
        in_maps.append({
            "slabg": np.ascontiguousarray(slabg),
            "catwh": catwh.astype(bf16),
            "zwh": zwh.astype(bf16),
            "whaug": whaug.astype(bf16),
            "sP": sP,
            "sPn": np.ascontiguousarray(-sP),
            "uw": np.ascontiguousarray(np.stack([u[r], w[r]])),
            "dsel": dsel,
            "gamma": np.ascontiguousarray(gamma[None, :]).astype(np.float32),
            "beta": np.ascontiguousarray(beta[None, :]).astype(np.float32),
            "w1t": np.ascontiguousarray(W1.T).astype(bf16),
            "b1": np.ascontiguousarray(b1[:, None]).astype(np.float32),
            "w2t": np.ascontiguousarray(W2.T).astype(bf16),
            "b2": np.ascontiguousarray(b2[:, None]).astype(np.float32),
            "w3t": np.ascontiguousarray(W3.T).astype(bf16),
            "b3": np.ascontiguousarray(b3[:, None]).astype(np.float32),
        })
    return in_maps


_NC_CACHE = {}


def kernel(x, adj, W_gat, a, gamma, beta, W1, b1, W2, b2, W3, b3,
           trace=False):
    from concourse.bass_utils import run_bass_kernel_spmd

    args = [np.asarray(t) for t in
            (x, adj, W_gat, a, gamma, beta, W1, b1, W2, b2, W3, b3)]
    in_maps = host_prep(*args)
    if "nc" not in _NC_CACHE:
        _NC_CACHE["nc"] = build_nc()
    nc = _NC_CACHE["nc"]
    res = run_bass_kernel_spmd(nc, in_maps, list(range(N_CORES)), trace=trace)
    out = np.concatenate([r["out"] for r in res.results], axis=0)
    if trace:
        kernel.last_results = res
    return out.astype(np.float32)


# revision 28
# speedup vs baseline: 1.3851x; 1.1772x over previous
"""Trainium2 Bass kernel for nn_MetaRL_LightGAT_BiACT (GAT + LayerNorm + MLP).

Strategy (8 NeuronCores, row-sharded, indicator-split formulation):

  exp(leaky_relu(s_i + s_j)) is exactly u_i*v_j when s_i+s_j > 0 and
  w_i*z_j otherwise, where u=exp(s), w=exp(0.2 s) (v=u, z=w over j).
  With c_ij = 1[s_i+s_j > 0] and A1 = adj*c, the GAT aggregation
  numerator (and denominator, via a ones column) becomes

     num_i = u_i * (A1 @ vWh)_i + w_i * ((adj @ zWh)_i - (A1 @ zWh)_i)

  i.e. two plain matmuls per j-chunk with 0/1 rhs masks -- no exp or
  leaky-relu over the N^2 data at all.

  Host precomputes a single pre-transposed fp16 slab
     slabG[j, i] = s_i + 4*(adj[i,j] - 1)
  from which BOTH masks fall out as one tensor_scalar each (4x DVE mode):
     A1 = (slabG + s_j) > 0        (adj=0 entries are < -2, never pass)
     A0 = slabG > -2               (recovers adj)

  Per j-chunk (128 j's x 1024 i's) on each core:
    DMA:  slabG chunk [128, 1024] fp16 (plain contiguous load)
    DVE:  A1 = ts(slabG add s_j, is_gt 0) -> bf16   (4x mode)
          A0 = ts(slabG is_gt -2)         -> bf16   (4x mode)
    PE:   accCat[0:128]  += [vWh | -zWh]_chunk^T @ A1   (bf16, 1 cyc/row)
          accCat[64:128] += zWh_chunk^T @ A0            (same PSUM bank;
                            accumulates zWh@(A0-A1) in rows 64..112)
  Epilogue: shift accCat[64:113] down via tiny DMA, combine with u/w,
  divide by denominator row, LayerNorm (f32), MLP 48->256->128->32 in
  bf16 on PE, transpose out.
"""

import sys

if "/opt/trn_rl_repo" not in sys.path:
    sys.path.insert(0, "/opt/trn_rl_repo")

import numpy as np

N = 8192
D_IN = 128
D_H = 48
D_OUT = 32
N_CORES = 8
ROWS = N // N_CORES          # 1024 rows (i) per core
P = 128                      # partitions
NEG_SLOPE = 0.2
EPS = 1e-5
MOFF = 60.0                  # mask offset folded into slabG


def build_nc(num_cores=N_CORES, rows=ROWS, n=N, slab_bufs=6, mask_bufs=6,
             reps=1, exp_chunks=28, lrelu_mode="act",
             stages="dma,cmp,mm,epi"):
    import concourse.bass as bass
    import concourse.mybir as mybir
    import concourse.tile as tile
    from concourse import bacc
    from concourse.masks import make_identity
    from contextlib import ExitStack

    f32 = mybir.dt.float32
    f16 = mybir.dt.float16
    bf16 = mybir.dt.bfloat16
    AF = mybir.ActivationFunctionType
    OP = mybir.AluOpType

    n_chunk = n // P             # j-chunks
    n_half = rows // 512         # 512-wide i halves

    st = {}
    for tok in stages.split(","):
        name, _, mult = tok.partition(":")
        st[name] = int(mult) if mult else 1
    nc = bacc.Bacc("TRN2", target_bir_lowering=False, debug=False,
                   num_devices=num_cores)

    slab_d = nc.dram_tensor("slabg", [n, rows], bf16, kind="ExternalInput").ap()
    catwh_d = nc.dram_tensor("catwh", [n, P], bf16, kind="ExternalInput").ap()
    zwh_d = nc.dram_tensor("zwh", [n, 64], bf16, kind="ExternalInput").ap()
    whaug_d = nc.dram_tensor("whaug", [n, 64], bf16, kind="ExternalInput").ap()
    sP_d = nc.dram_tensor("sP", [P, n_chunk], f32, kind="ExternalInput").ap()
    sPn_d = nc.dram_tensor("sPn", [P, n_chunk], f32, kind="ExternalInput").ap()
    uw_d = nc.dram_tensor("uw", [2, rows], f32, kind="ExternalInput").ap()
    dsel_d = nc.dram_tensor("dsel", [P, 1], f32, kind="ExternalInput").ap()
    gamma_d = nc.dram_tensor("gamma", [1, D_H], f32, kind="ExternalInput").ap()
    beta_d = nc.dram_tensor("beta", [1, D_H], f32, kind="ExternalInput").ap()
    w1t_d = nc.dram_tensor("w1t", [D_H, 256], bf16, kind="ExternalInput").ap()
    b1_d = nc.dram_tensor("b1", [256, 1], f32, kind="ExternalInput").ap()
    w2t_d = nc.dram_tensor("w2t", [256, 128], bf16, kind="ExternalInput").ap()
    b2_d = nc.dram_tensor("b2", [128, 1], f32, kind="ExternalInput").ap()
    w3t_d = nc.dram_tensor("w3t", [128, D_OUT], bf16, kind="ExternalInput").ap()
    b3_d = nc.dram_tensor("b3", [D_OUT, 1], f32, kind="ExternalInput").ap()
    out_d = nc.dram_tensor("out", [rows, D_OUT], f32, kind="ExternalOutput").ap()

    with ExitStack() as ctx:
        tc = ctx.enter_context(tile.TileContext(nc))
        singles = ctx.enter_context(tc.tile_pool(name="singles", bufs=1))
        slabp = ctx.enter_context(tc.tile_pool(name="slabp", bufs=slab_bufs))
        maskp = ctx.enter_context(tc.tile_pool(name="maskp", bufs=mask_bufs))
        hp = ctx.enter_context(tc.tile_pool(name="hp", bufs=2))

        # ---- resident small tensors ----
        catwh_sb = singles.tile([P, n_chunk, P], bf16)
        nc.sync.dma_start(catwh_sb, catwh_d.rearrange("(c p) m -> p c m", p=P))
        zwh_sb = singles.tile([P, n_chunk, 64], bf16)
        nc.sync.dma_start(zwh_sb, zwh_d.rearrange("(c p) m -> p c m", p=P))
        sP_sb = singles.tile([P, n_chunk], f32)
        nc.sync.dma_start(sP_sb, sP_d)
        sPn_sb = singles.tile([P, n_chunk], f32)
        nc.sync.dma_start(sPn_sb, sPn_d)
        sP2_sb = singles.tile([P, n_chunk], f32)
        nc.scalar.activation(sP2_sb, sP_sb, AF.Copy, scale=NEG_SLOPE)
        whaug_sb = singles.tile([P, n_chunk, 64], bf16)
        nc.sync.dma_start(whaug_sb, whaug_d.rearrange("(c p) m -> p c m", p=P))
        # u replicated over partitions 0..63, w over all 128 (used at 64:113)
        u_rep = singles.tile([64, rows], f32)
        nc.sync.dma_start(u_rep, uw_d[0:1, :].partition_broadcast(64)
                          .rearrange("p one r -> p (one r)"))
        w_rep = singles.tile([P, rows], f32)
        nc.sync.dma_start(w_rep, uw_d[1:2, :].partition_broadcast(P)
                          .rearrange("p one r -> p (one r)"))
        gammaC = singles.tile([D_H, 1], f32)
        nc.sync.dma_start(gammaC, gamma_d.rearrange("one d -> d one"))
        betaC = singles.tile([D_H, 1], f32)
        nc.sync.dma_start(betaC, beta_d.rearrange("one d -> d one"))
        w1t_sb = singles.tile([D_H, 256], bf16)
        nc.sync.dma_start(w1t_sb, w1t_d)
        w2t_sb = singles.tile([P, 2, 128], bf16)
        nc.sync.dma_start(w2t_sb, w2t_d.rearrange("(m p) k -> p m k", p=P))
        w3t_sb = singles.tile([P, D_OUT], bf16)
        nc.sync.dma_start(w3t_sb, w3t_d)
        b1_sb = singles.tile([P, 2], f32)
        nc.sync.dma_start(b1_sb, b1_d.rearrange("(m p) one -> p (m one)", p=P))
        b2_sb = singles.tile([P, 1], f32)
        nc.sync.dma_start(b2_sb, b2_d)
        b3_sb = singles.tile([D_OUT, 1], f32)
        nc.sync.dma_start(b3_sb, b3_d)
        eps_sb = singles.tile([P, 1], f32)
        nc.vector.memset(eps_sb, EPS)
        dsel_sb = singles.tile([P, 1], f32)
        nc.sync.dma_start(dsel_sb, dsel_d)
        ones1x48 = singles.tile([1, D_H], f32)
        nc.vector.memset(ones1x48, 1.0)
        ones48 = singles.tile([D_H, 1], bf16)
        nc.vector.memset(ones48, 1.0)
        ident = singles.tile([P, P], f32)
        make_identity(nc, ident)

        def bcast_sb(dst, src_row, parts, eng=None):
            src = bass.AP(tensor=src_row.tensor, offset=src_row.offset,
                          ap=[src_row.ap[0], [0, parts], src_row.ap[1]])
            dst3 = bass.AP(tensor=dst.tensor, offset=dst.offset,
                           ap=[dst.ap[0], [1, 1], dst.ap[1]])
            (eng or nc.sync).dma_start(dst3, src)

        slab_r = slab_d.rearrange("(c p) i -> p c i", p=P)

        # chunk classes: E (exp path on ACT, 1 matmul arm) / D (indicator
        # path on DVE, 2 matmul arms). Interleave E chunks for overlap.
        n_e = min(exp_chunks * n_chunk // 64, n_chunk)
        is_e = [(cc * n_e // n_chunk) != ((cc + 1) * n_e // n_chunk)
                for cc in range(n_chunk)]
        e_idx = [cc for cc in range(n_chunk) if is_e[cc]]
        d_idx = [cc for cc in range(n_chunk) if not is_e[cc]]
        def emit_phase2(hs, rep):
          # MLP head in transposed layout (bf16); deferred into the next
          # rep's chunk stream so the LN chain doesn't block fresh matmuls
          with tc.tile_pool(name=f"mlpp{rep}", bufs=1, space="PSUM") as mlpp:
            for h in range(len(hs)):
                h1 = hp.tile([P, 2, 512], bf16, tag="h1")
                for m in range(2):
                    m1 = mlpp.tile([P, 512], f32, tag="mm", bufs=2)
                    nc.tensor.matmul(m1, lhsT=w1t_sb[:, m * P:(m + 1) * P],
                                     rhs=hs[h], start=True, stop=True)
                    nc.scalar.activation(h1[:, m, :], m1, AF.Relu,
                                         bias=b1_sb[:, m:m + 1])
                m2 = mlpp.tile([P, 512], f32, tag="mm", bufs=2)
                for m in range(2):
                    nc.tensor.matmul(m2, lhsT=w2t_sb[:, m, :],
                                     rhs=h1[:, m, :],
                                     start=(m == 0), stop=(m == 1))
                h2 = hp.tile([P, 512], bf16, tag="h2")
                nc.scalar.activation(h2, m2, AF.Relu, bias=b2_sb)
                m3 = mlpp.tile([D_OUT, 512], f32, tag="mm", bufs=2)
                nc.tensor.matmul(m3, lhsT=w3t_sb, rhs=h2,
                                 start=True, stop=True)
                h3 = hp.tile([D_OUT, 512], f32, tag="h3")
                nc.scalar.activation(h3, m3, AF.Identity, bias=b3_sb)
                for k in range(4):
                    ko = h * 4 + k
                    m4 = mlpp.tile([P, D_OUT], f32, tag="m4")
                    nc.tensor.transpose(m4, h3[:, k * P:(k + 1) * P],
                                        ident[0:D_OUT, 0:D_OUT])
                    ob = hp.tile([P, D_OUT], f32, tag="ob")
                    nc.vector.tensor_copy(ob, m4)
                    nc.gpsimd.dma_start(out_d[ko * P:(ko + 1) * P, :], ob)

        pend = []
        for rep in range(reps):
          with tc.tile_pool(name=f"accp{rep}", bufs=2, space="PSUM") as accp:
            acc = [accp.tile([P, 512], f32, tag="acc", name=f"acc{h}")
                   for h in range(n_half)]
            accEp = accp.tile([P, 512], f32, tag="accE", name="accE")
            accE = [accEp[64 * h:64 * h + 64, :] for h in range(n_half)]
            for cc in range(n_chunk):
                if cc == max(1, min(8, n_chunk // 8)) and pend:
                    emit_phase2(*pend.pop())
                slab = slabp.tile([P, rows], bf16, tag="slab")
                for _m in range(st.get("dma", 0)):
                    nc.sync.dma_start(slab, slab_r[:, cc, :])
                n_mm = st.get("mm", 0)
                if is_e[cc]:
                    te = maskp.tile([P, rows], bf16, tag="a1")
                    pe_ = maskp.tile([P, rows], bf16, tag="a0")
                    for _m in range(st.get("cmp", 0)):
                        if lrelu_mode == "act":
                            nc.scalar.activation(te, slab, AF.Prelu,
                                                 bias=sP_sb[:, cc:cc + 1],
                                                 alpha=NEG_SLOPE)
                            nc.scalar.activation(pe_, te, AF.Exp)
                        else:  # exp(leaky(x)) == max(exp(x), exp(0.2 x))
                            nc.scalar.activation(te, slab, AF.Exp,
                                                 scale=NEG_SLOPE,
                                                 bias=sP2_sb[:, cc:cc + 1])
                            nc.scalar.activation(pe_, slab, AF.Exp,
                                                 bias=sP_sb[:, cc:cc + 1])
                            nc.vector.tensor_tensor(pe_, pe_, te, OP.max)
                    for _m in range(n_mm):
                        for h in range(n_half):
                            sl = slice(h * 512, (h + 1) * 512)
                            nc.tensor.matmul(
                                accE[h],
                                lhsT=whaug_sb[:, cc, :],
                                rhs=pe_[:, sl],
                                start=(cc == e_idx[0] and _m == 0),
                                stop=(cc == e_idx[-1] and _m == n_mm - 1
                                      and not d_idx),
                                skip_group_check=True)
                    continue
                a1 = maskp.tile([P, rows], bf16, tag="a1")
                a0 = maskp.tile([P, rows], bf16, tag="a0")
                for _m in range(st.get("cmp", 0)):
                    nc.vector.tensor_scalar(a1, slab, sPn_sb[:, cc:cc + 1],
                                            None, OP.is_gt)
                    nc.vector.tensor_scalar(a0, slab, -(MOFF / 2), None,
                                            OP.is_gt)
                for _m in range(n_mm):
                    for h in range(n_half):
                        sl = slice(h * 512, (h + 1) * 512)
                        nc.tensor.matmul(
                            acc[h][:, :],
                            lhsT=catwh_sb[:, cc, :],
                            rhs=a1[:, sl],
                            start=(cc == d_idx[0] and _m == 0), stop=False,
                            skip_group_check=True)
                    for h in range(n_half):
                        sl = slice(h * 512, (h + 1) * 512)
                        nc.tensor.matmul(
                            acc[h][64:128, :],
                            lhsT=zwh_sb[:, cc, :],
                            rhs=a0[:, sl],
                            start=False,
                            stop=(cc == d_idx[-1] and _m == n_mm - 1),
                            skip_group_check=True)

            # ---- epilogue phase 1: combine, divide, LayerNorm ----
            hs = []
            do_epi = st.get("epi", 0) > 0 and st.get("mm", 0) > 0
            for h in range(n_half if do_epi else 0):
                sl = slice(h * 512, (h + 1) * 512)
                # combine: acc rows 0:49 hold vWh@A1 (u side), rows 64:113
                # hold zWh@(A0-A1) (w side). Weight each in place in PSUM,
                # bounce to SBUF, shift the w side down 64 partitions via
                # DMA, add, then divide by the denominator row.
                numT = hp.tile([49, 512], f32, tag="numT")
                if d_idx:
                    nc.vector.tensor_tensor(acc[h][64:113, :],
                                            acc[h][64:113, :],
                                            w_rep[64:113, sl], OP.mult)
                    nc.vector.tensor_tensor(acc[h][0:49, :], acc[h][0:49, :],
                                            u_rep[0:49, sl], OP.mult)
                    comb = hp.tile([P, 512], f32, tag="comb")
                    nc.vector.tensor_copy(comb, acc[h][:, :])
                    # fold the 64->0 partition shift into the accE group:
                    # accE[0:49] += comb[64:113] via identity-slice matmul
                    nc.tensor.matmul(accE[h][0:49, :],
                                     lhsT=ident[:, 64:113], rhs=comb,
                                     start=(not e_idx), stop=True,
                                     skip_group_check=True)
                    nc.vector.tensor_tensor(numT, comb[0:49, :],
                                            accE[h][0:49, :], OP.add)
                else:
                    nc.vector.tensor_copy(numT, accE[h][0:49, :])
                den0 = hp.tile([1, 512], f32, tag="den0")
                nc.gpsimd.dma_start(den0, numT[48:49, :])
                rec = hp.tile([1, 512], f32, tag="rec")
                nc.vector.reciprocal_approx_fast(rec, den0)
                rbc = hp.tile([D_H, 512], f32, tag="rbc")
                bcast_sb(rbc, rec[0:1, :], D_H, eng=nc.gpsimd)
                hT = hp.tile([D_H, 512], bf16, tag="hT", bufs=n_half)
                nc.vector.tensor_tensor(hT, numT[0:D_H, :], rbc, OP.mult)
                sq = hp.tile([D_H, 512], bf16, tag="sq")
                nc.scalar.activation(sq, hT, AF.Square)
                stats = accp.tile([33, 512], f32, tag="stats",
                                  name="stats", bufs=1)
                ssum = stats[0:1, :]
                nc.tensor.matmul(ssum, lhsT=ones48, rhs=hT,
                                 start=True, stop=True)
                ssq = stats[32:33, :]
                nc.tensor.matmul(ssq, lhsT=ones48, rhs=sq,
                                 start=True, stop=True)
                mean = hp.tile([1, 512], f32, tag="mean")
                nc.vector.tensor_scalar(mean, ssum, 1.0 / D_H, None, OP.mult)
                var = hp.tile([1, 512], f32, tag="var")
                nc.vector.tensor_scalar(var, ssq, 1.0 / D_H, EPS,
                                        OP.mult, OP.add)
                msq = hp.tile([1, 512], f32, tag="msq")
                nc.vector.tensor_tensor(msq, mean, mean, OP.mult)
                nc.vector.tensor_tensor(var, var, msq, OP.subtract)
                std = hp.tile([1, 512], f32, tag="std")
                nc.scalar.activation(std, var, AF.Sqrt)
                rstd = hp.tile([1, 512], f32, tag="rstd")
                nc.vector.reciprocal_approx_fast(rstd, std)
                mbc = hp.tile([D_H, 512], f32, tag="mbc")
                bcast_sb(mbc, mean[0:1, :], D_H, eng=nc.gpsimd)
                sbc = hp.tile([D_H, 512], f32, tag="sbc")
                bcast_sb(sbc, rstd[0:1, :], D_H, eng=nc.gpsimd)
                nc.vector.tensor_tensor(hT, hT, mbc, OP.subtract)
                nc.vector.tensor_tensor(hT, hT, sbc, OP.mult)
                hTb = hp.tile([D_H, 512], bf16, tag="hTb",
                              bufs=2 * n_half)
                nc.vector.tensor_scalar(hTb, hT, gammaC, betaC,
                                        OP.mult, OP.add)
                hs.append(hTb)
          if hs:
              pend.append((hs, rep))
        while pend:
            emit_phase2(*pend.pop())

    nc.compile()
    return nc


def host_prep(x, adj, W_gat, a, gamma, beta, W1, b1, W2, b2, W3, b3,
              num_cores=N_CORES):
    import ml_dtypes

    bf16 = ml_dtypes.bfloat16
    n = x.shape[0]
    rows = n // num_cores
    n_chunk = n // P
    Wh = (x.astype(np.float32) @ W_gat.T.astype(np.float32))
    s = (Wh @ a.T.astype(np.float32)).ravel().astype(np.float32)
    assert np.abs(s).max() < MOFF / 2 - 0.1, "s out of slab-offset range"
    u = np.exp(s).astype(np.float32)          # exp(s)
    w = np.exp(NEG_SLOPE * s).astype(np.float32)
    # catwh: [vWh(48) v 0*15 | -zWh(48) -z 0*15]
    catwh = np.zeros((n, P), np.float32)
    catwh[:, 0:D_H] = u[:, None] * Wh
    catwh[:, D_H] = u
    catwh[:, 64:64 + D_H] = -(w[:, None] * Wh)
    catwh[:, 64 + D_H] = -w
    zwh = np.zeros((n, 64), np.float32)
    zwh[:, 0:D_H] = w[:, None] * Wh
    zwh[:, D_H] = w
    whaug = np.zeros((n, 64), np.float32)
    whaug[:, 0:D_H] = Wh
    whaug[:, D_H] = 1.0
    sP = np.ascontiguousarray(s.reshape(n_chunk, P).T)
    dsel = np.zeros((P, 1), np.float32)
    dsel[D_H, 0] = 1.0
    dsel[64 + D_H, 0] = 1.0
    in_maps = []
    for c in range(num_cores):
        r = slice(c * rows, (c + 1) * rows)
        slabg = (s[r][None, :] +
                 MOFF * (adj[r].T.astype(np.float32) - 1.0)
                 ).astype(bf16)
        in_maps.append({
            "slabg": np.ascontiguousarray(slabg),
            "catwh": catwh.astype(bf16),
            "zwh": zwh.astype(bf16),
            "whaug": whaug.astype(bf16),
            "sP": sP,
            "sPn": np.ascontiguousarray(-sP),
            "uw": np.ascontiguousarray(np.stack([u[r], w[r]])),
            "dsel": dsel,
            "gamma": np.ascontiguousarray(gamma[None, :]).astype(np.float32),
            "beta": np.ascontiguousarray(beta[None, :]).astype(np.float32),
            "w1t": np.ascontiguousarray(W1.T).astype(bf16),
            "b1": np.ascontiguousarray(b1[:, None]).astype(np.float32),
            "w2t": np.ascontiguousarray(W2.T).astype(bf16),
            "b2": np.ascontiguousarray(b2[:, None]).astype(np.float32),
            "w3t": np.ascontiguousarray(W3.T).astype(bf16),
            "b3": np.ascontiguousarray(b3[:, None]).astype(np.float32),
        })
    return in_maps


_NC_CACHE = {}


def kernel(x, adj, W_gat, a, gamma, beta, W1, b1, W2, b2, W3, b3,
           trace=False):
    from concourse.bass_utils import run_bass_kernel_spmd

    args = [np.asarray(t) for t in
            (x, adj, W_gat, a, gamma, beta, W1, b1, W2, b2, W3, b3)]
    in_maps = host_prep(*args)
    if "nc" not in _NC_CACHE:
        _NC_CACHE["nc"] = build_nc()
    nc = _NC_CACHE["nc"]
    res = run_bass_kernel_spmd(nc, in_maps, list(range(N_CORES)), trace=trace)
    out = np.concatenate([r["out"] for r in res.results], axis=0)
    if trace:
        kernel.last_results = res
    return out.astype(np.float32)
